# revision 40
# baseline (speedup 1.0000x reference)
"""Multi-head cross-attention Trainium2 kernel (8-core SPMD).

Sharding: 2 batch groups x 4 cores. Core c handles batch b = c // 4 and
heads [4*(c%4), 4*(c%4)+4). Each core computes its 4 heads' attention
output and a partial output projection (row-sharded Wp); the host sums
the 4 partials per batch (the all-reduce step of tensor parallelism).

All matmul operands are fp16 (PE runs fp16 at full rate), accumulation
fp32 in PSUM. Structure (per core, "mh" = head pair, 2 per core):

  Q^T[d,t] = Wq4.T @ xT (+bias)
  K^T[d,s] = Wk4.T @ eT (+bias)   -- lhsT for the QK matmuls
  V[s,d]   = eT.T @ Wv4 (+bias via a K=1 ones-row matmul) -- computed
             directly in [s,d] layout (no PE transposes); DVE strided
             copy drops it into VA[s, v0|1|v1|1] (softmax-denominator
             ones columns ride along).
  attention loop (per mh, tq=512 queries, s-tile=128 keys):
    att[s, h0q|h1q] = joint [128,1024] PSUM tile; the two QK matmuls
    (K=64 each) auto-row-tile into array halves and run concurrently.
    exp: ACT (exact) for 14/16 s-tiles; DVE Schraudolph fp16-bitcast
    (i16 = att*A + B) for s in {3, 11} to keep ACT off the critical
    path. U_aug[65,tq] += V_aug.T @ E accumulates outputs + denoms.
  normalize: denom row -> fp16 -> PE ones-broadcast -> fast reciprocal
    -> UN = U * (1/d); deferred into the next tq's PE slack.
  outproj: per t-tile, 4 accumulating matmuls -> fp16 evac -> DMA.
    y is fp16; the host sums partials in fp32.

Deferred work (next pair's Q/K, normalize, outproj) is paced into the
attention loop with per-iteration PE/DVE cost budgets; during (mh0,
tq0) the K-chunks and V-tiles stream in just ahead of their first use
so attention starts as soon as the first chunks land (~17us).

PSUM: att 2x2 banks + uh 2x1 + kv 2x1 (Q/K/V psums, denom bcast,
outproj all share the kv tag) = 8 banks.
"""

import os
import numpy as np
from contextlib import ExitStack
from collections import deque

import concourse.bass as bass
import concourse.bacc as bacc
import concourse.tile as tile
from concourse import mybir
from concourse.bass_utils import run_bass_kernel_spmd

F32 = mybir.dt.float32
F16 = mybir.dt.float16
I16 = mybir.dt.int16
AF = mybir.ActivationFunctionType
ALU = mybir.AluOpType

B, T, S, C = 2, 2048, 2048, 1024
H, HD = 16, 64
NCORES = 8
HPC = 4            # heads per core
MHN = 2            # head-pairs per core
KC = C // 128      # 8 contraction tiles
STILES = S // 128  # 16
TTILES = T // 128  # 16
TQN = 4            # t-quarters of 512
SCALE = 1.0 / np.sqrt(C)

# Schraudolph fp16 exp on DVE: i16 = round(att*EXP_A + EXP_B) bitcast
# to fp16 ~= exp(att*SCALE). C-shift -24 centers the mantissa-linear
# error (RMS ~1.05%). Only the s-tiles in DVE_EXP_TILES take this path.
EXP_A = float(1024.0 * np.log2(np.e) * SCALE)
EXP_B = float(15 * 1024 - 24)
DVE_EXP_TILES = (3, 11)

LAST_RESULTS = None
_NC_CACHE = None


def _build_nc():
    nc = bacc.Bacc()

    # all inputs host-pre-swizzled to partition-major [128, ...] layouts
    xT = nc.declare_dram_parameter("xT", [128, KC, T], F16, isOutput=False)
    eT = nc.declare_dram_parameter("eT", [128, KC, S], F16, isOutput=False)
    Wq4 = nc.declare_dram_parameter("Wq4", [128, KC, 256], F16, isOutput=False)
    Wk4 = nc.declare_dram_parameter("Wk4", [128, KC, 256], F16, isOutput=False)
    Wv4 = nc.declare_dram_parameter("Wv4", [128, KC, 256], F16, isOutput=False)
    b6 = nc.declare_dram_parameter("b6", [128, 6], F32, isOutput=False)
    bvr = nc.declare_dram_parameter("bvr", [1, 256], F16, isOutput=False)
    WpT4 = nc.declare_dram_parameter("WpT4", [128, 2, C], F16, isOutput=False)
    y = nc.declare_dram_parameter("y", [T, C], F16, isOutput=True)


    with tile.TileContext(nc) as tc, ExitStack() as ctx:
        consts = ctx.enter_context(tc.tile_pool(name="consts", bufs=1))
        wpool = ctx.enter_context(tc.tile_pool(name="wts", bufs=1))
        qkvp = ctx.enter_context(tc.tile_pool(name="qkvt", bufs=2))
        vap = ctx.enter_context(tc.tile_pool(name="vaug", bufs=2))
        epool = ctx.enter_context(tc.tile_pool(name="esb", bufs=6))
        unp = ctx.enter_context(tc.tile_pool(name="unorm", bufs=2))
        usbp = ctx.enter_context(tc.tile_pool(name="usb", bufs=3))
        dnp = ctx.enter_context(tc.tile_pool(name="denom", bufs=2))
        ysbp = ctx.enter_context(tc.tile_pool(name="ysb", bufs=3))
        psp = ctx.enter_context(tc.tile_pool(name="ps", bufs=2, space="PSUM"))

        # ---- constants ----
        # ones65[64:65, :] is the denom-broadcast lhsT; it lives on
        # partition 64 to match the usb denom row's base partition.
        ones65 = consts.tile([65, 64], F16, tag="ones65", name="ones65")
        nc.gpsimd.memset(ones65, 1.0)
        onesr = consts.tile([1, 128], F16, tag="onesr", name="onesr")
        nc.gpsimd.memset(onesr, 1.0)
        b6sb = consts.tile([128, 6], F32, tag="b6", name="b6sb")
        nc.sync.dma_start(out=b6sb, in_=b6[:, :])
        bvsb = consts.tile([1, 256], F16, tag="bvr", name="bvsb")
        nc.sync.dma_start(out=bvsb, in_=bvr[:, :])
        bsb = {"q": b6sb[:, 0:2], "k": b6sb[:, 2:4]}

        # ---- input DMAs, ordered by first consumer ----
        wsb = {}
        wsb["q"] = wpool.tile([128, KC, 256], F16, tag="wq", name="wqsb")
        nc.sync.dma_start(out=wsb["q"], in_=Wq4[:, :, :])
        xt_sb = wpool.tile([128, KC, T], F16, tag="xt")
        for k in range(KC):
            nc.sync.dma_start(out=xt_sb[:, k, :], in_=xT[:, k, :])
        for nm, dram in (("k", Wk4), ("v", Wv4)):
            t_ = wpool.tile([128, KC, 256], F16, tag=f"w{nm}", name=f"w{nm}sb")
            nc.sync.dma_start(out=t_, in_=dram[:, :, :])
            wsb[nm] = t_
        # eT lands s-chunk-major so K/V s-tiles can start early
        et_sb = wpool.tile([128, KC, S], F16, tag="et")
        for sc in range(4):
            csl = slice(sc * 512, (sc + 1) * 512)
            for k in range(KC):
                nc.sync.dma_start(out=et_sb[:, k, csl], in_=eT[:, k, csl])
        wpt = wpool.tile([128, 2, C], F16, tag="wpt")
        nc.sync.dma_start(out=wpt, in_=WpT4[:, :, :])

        QT = [qkvp.tile([128, T], F16, tag="qt", name=f"QT{i}")
              for i in range(MHN)]
        KT = [qkvp.tile([128, S], F16, tag="kt", name=f"KT{i}")
              for i in range(MHN)]
        VA = [vap.tile([128, STILES, 130], F16, tag="va", name=f"va{i}")
              for i in range(MHN)]
        for i in range(MHN):
            nc.gpsimd.memset(VA[i][:, :, 64:65], 1.0)
            nc.gpsimd.memset(VA[i][:, :, 129:130], 1.0)
        UN = [unp.tile([128, T], F16, tag="un", name=f"UN{i}")
              for i in range(MHN)]

        # Work items are (pe_ns, dve_ns, closure) for budgeted pacing.
        def q_pass(mh, half):
            tqs = (half * 2, half * 2 + 1)
            state = {}

            def mk_alloc():
                state["ps"] = {tq: psp.tile([128, 512], F32, tag="kv",
                                            name=f"qtps{mh}_{tq}")
                               for tq in tqs}

            yield (0, 0, mk_alloc)
            for k in range(KC):
                def mk_k(k=k):
                    lhsT = wsb["q"][:, k, mh * 128:(mh + 1) * 128]
                    for tq in tqs:
                        nc.tensor.matmul(state["ps"][tq], lhsT,
                                         xt_sb[:, k, tq * 512:(tq + 1) * 512],
                                         start=(k == 0), stop=(k == KC - 1))

                yield (430, 0, mk_k)

            def mk_evac():
                for tq in tqs:
                    nc.vector.tensor_scalar_add(
                        out=QT[mh][:, tq * 512:(tq + 1) * 512],
                        in0=state["ps"][tq], scalar1=bsb["q"][:, mh:mh + 1])

            yield (0, 1450, mk_evac)

        def k_work(mh, sc):
            csl = slice(sc * 512, (sc + 1) * 512)
            state = {}

            def mk_alloc():
                state["ps"] = psp.tile([128, 512], F32, tag="kv",
                                       name=f"ktps{mh}_{sc}")

            yield (0, 0, mk_alloc)

            for k in range(KC):
                def mk_k(k=k):
                    nc.tensor.matmul(state["ps"],
                                     wsb["k"][:, k, mh * 128:(mh + 1) * 128],
                                     et_sb[:, k, csl],
                                     start=(k == 0), stop=(k == KC - 1))

                yield (215, 0, mk_k)

            def mk_evac():
                nc.vector.tensor_scalar_add(out=KT[mh][:, csl],
                                            in0=state["ps"],
                                            scalar1=bsb["k"][:, mh:mh + 1])

            yield (0, 730, mk_evac)

        def v_work(j):
            """V[s,d] for s-tile j, all 4 heads at once; bias rides as a
            K=1 accumulating matmul; strided copy into VA (skips the
            ones columns)."""
            state = {}

            def mk_k4(k0):
                if k0 == 0:
                    state["ps"] = psp.tile([128, 512], F32, tag="kv",
                                           name=f"vps{j}")
                vp = state["ps"][:, 0:256]
                for k in range(k0, k0 + 4):
                    nc.tensor.matmul(vp, et_sb[:, k, j * 128:(j + 1) * 128],
                                     wsb["v"][:, k, 0:256],
                                     start=(k == 0), stop=False)
                if k0 == 4:
                    nc.tensor.matmul(vp, onesr, bvsb[:, :],
                                     start=False, stop=True)

            def mk_evac():
                vp = state["ps"]
                for mh in range(MHN):
                    dst = VA[mh][:, j, 0:130].rearrange(
                        "p (b d) -> p b d", b=2)[:, :, 0:64]
                    src = vp[:, mh * 128:(mh + 1) * 128].rearrange(
                        "p (b d) -> p b d", b=2)
                    nc.vector.tensor_copy(dst, src)

            yield (440, 0, lambda: mk_k4(0))
            yield (550, 0, lambda: mk_k4(4))
            yield (0, 470, mk_evac)

        y_r = y.rearrange("(tt p) o -> tt p o", p=128)

        def outproj_work(tq):
            for j in range(4):
                t = tq * 4 + j
                for n in range(2):
                    def mk(t=t, n=n):
                        nsl = slice(n * 512, (n + 1) * 512)
                        y_ps = psp.tile([128, 512], F32, tag="kv",
                                        name=f"yps{t}_{n}")
                        for mh in range(MHN):
                            nc.tensor.matmul(
                                y_ps, UN[mh][:, t * 128:(t + 1) * 128],
                                wpt[:, mh, nsl],
                                start=(mh == 0), stop=(mh == MHN - 1))
                        ysb = ysbp.tile([128, 512], F16, tag="ysb",
                                        name=f"ysb{t}_{n}")
                        nc.vector.tensor_copy(ysb, y_ps)
                        nc.sync.dma_start(out=y_r[t][:, nsl], in_=ysb)

                    yield (440, 730, mk)

        def normalize_work(mh, tq, uh0, uh1):
            """Evac U, broadcast the denom row (rides in uh row 64) via a
            fp16 ones matmul, reciprocal, divide."""
            qsl = slice(tq * 512, (tq + 1) * 512)
            state = {}

            def mk_evac():
                usb = usbp.tile([65, 1024], F16, tag="usb",
                                name=f"usb{mh}_{tq}")
                nc.vector.tensor_copy(usb[:, 0:512], uh0)
                nc.vector.tensor_copy(usb[:, 512:1024], uh1)
                state["usb"] = usb

            def mk_bcast():
                usb = state["usb"]
                bc = [psp.tile([128, 512], F32, tag="kv",
                               name=f"bcps{mh}_{tq}_{i}") for i in range(2)]
                state["bc"] = bc
                for i in range(2):
                    nc.tensor.matmul(bc[i][0:64, :], ones65[64:65, :],
                                     usb[64:65, i * 512:(i + 1) * 512],
                                     start=True, stop=True)

            def mk_div():
                usb, bc = state["usb"], state["bc"]
                rbc = dnp.tile([64, 1024], F32, tag="rbc", bufs=2,
                               name=f"rbc{mh}_{tq}")
                nc.vector.reciprocal_approx_fast(rbc[:, 0:512], bc[0][0:64, :])
                nc.vector.reciprocal_approx_fast(rbc[:, 512:1024],
                                                 bc[1][0:64, :])
                nc.vector.tensor_mul(UN[mh][0:64, qsl], usb[0:64, 0:512],
                                     rbc[:, 0:512])
                tmp1 = dnp.tile([64, 512], F16, tag="tmp1", bufs=2,
                                name=f"tmp1_{mh}_{tq}")
                nc.vector.tensor_mul(tmp1, usb[0:64, 512:1024],
                                     rbc[:, 512:1024])
                nc.gpsimd.dma_start(out=UN[mh][64:128, qsl], in_=tmp1)

            yield (0, 1300, mk_evac)
            yield (430, 0, mk_bcast)
            yield (0, 2600, mk_div)

        fast = deque()
        bulk = deque()
        stream = deque()

        def attention(mh):
            for tq in range(TQN):
                qsl = slice(tq * 512, (tq + 1) * 512)
                uh0 = psp.tile([65, 512], F32, tag="uh", name=f"uh0_{mh}_{tq}")
                uh1 = psp.tile([65, 512], F32, tag="uh", name=f"uh1_{mh}_{tq}")
                prev_av = None
                for s in range(STILES):
                    att = psp.tile([128, 1024], F32, tag="att",
                                   name=f"att_{mh}_{tq}_{s}")
                    ssl = slice(s * 128, (s + 1) * 128)
                    nc.tensor.matmul(att[:, 0:512], KT[mh][0:64, ssl],
                                     QT[mh][0:64, qsl], start=True, stop=True)
                    nc.tensor.matmul(att[:, 512:1024], KT[mh][64:128, ssl],
                                     QT[mh][64:128, qsl], start=True, stop=True)
                    ej = epool.tile([128, 1024], F16, tag="e",
                                    name=f"e_{mh}_{tq}_{s}")
                    if s in DVE_EXP_TILES:
                        nc.vector.tensor_scalar(
                            out=ej[:, :].bitcast(I16), in0=att,
                            scalar1=EXP_A, scalar2=EXP_B,
                            op0=ALU.mult, op1=ALU.add)
                    else:
                        nc.scalar.activation(ej, att, AF.Exp,
                                             scale=float(SCALE))
                    if prev_av is not None:
                        prev_av()
                    # paced deferred work: stream items (first tq only)
                    # hard-popped; otherwise budgeted fast/bulk pops
                    if stream:
                        npop = 0
                        while stream and npop < 6:
                            stream.popleft()[2]()
                            npop += 1
                    else:
                        pe_c = dve_c = npop = 0
                        while fast or bulk:
                            q = fast if fast else bulk
                            p, d, fn = q[0]
                            if npop and (pe_c + p > 620 or dve_c + d > 950):
                                break
                            q.popleft()
                            fn()
                            pe_c += p
                            dve_c += d
                            npop += 1

                    def mk_av(s=s, ej=ej, uh0=uh0, uh1=uh1):
                        nc.tensor.matmul(uh0, VA[mh][:, s, 0:65], ej[:, 0:512],
                                         start=(s == 0), stop=(s == STILES - 1))
                        nc.tensor.matmul(uh1, VA[mh][:, s, 65:130],
                                         ej[:, 512:1024],
                                         start=(s == 0), stop=(s == STILES - 1))
                    prev_av = mk_av
                prev_av()
                ngen = normalize_work(mh, tq, uh0, uh1)
                next(ngen)[2]()  # usb evac inline: frees uh slots promptly
                fast.extend(ngen)
                if mh == MHN - 1:
                    fast.extend(outproj_work(tq))

        # ---- schedule ----
        # Eager: Q pair 0 half 0 (tq0/tq1), first K chunk, first 2 V
        # tiles. The rest of KV streams into (mh0, tq0)'s loop just
        # ahead of first use; mh1's Q/K drains later as bulk.
        for it in q_pass(0, 0):
            it[2]()
        for it in k_work(0, 0):
            it[2]()
        for j in (0, 1):
            for it in v_work(j):
                it[2]()
        stream.extend(v_work(2))
        stream.extend(v_work(3))
        stream.extend(k_work(0, 1))
        for j in (4, 5):
            stream.extend(v_work(j))
        stream.extend(k_work(0, 2))
        for j in (6, 7, 8, 9):
            stream.extend(v_work(j))
        stream.extend(k_work(0, 3))
        for j in range(10, 16):
            stream.extend(v_work(j))

        bulk.extend(q_pass(0, 1))
        bulk.extend(q_pass(1, 0))
        bulk.extend(q_pass(1, 1))
        for sc in range(4):
            bulk.extend(k_work(1, sc))

        attention(0)
        attention(1)
        for q in (stream, fast, bulk):
            while q:
                q.popleft()[2]()

    nc.compile()
    return nc


def _get_nc():
    global _NC_CACHE
    if _NC_CACHE is None:
        _NC_CACHE = _build_nc()
    return _NC_CACHE


def make_in_maps(e, x, Wq, bq, Wk, bk, Wv, bv, Wp):
    e = np.asarray(e, dtype=np.float32)
    x = np.asarray(x, dtype=np.float32)
    Wq, bq = np.asarray(Wq, np.float32), np.asarray(bq, np.float32)
    Wk, bk = np.asarray(Wk, np.float32), np.asarray(bk, np.float32)
    Wv, bv = np.asarray(Wv, np.float32), np.asarray(bv, np.float32)
    Wp = np.asarray(Wp, np.float32)

    def swiz(a2d):  # [C, N] -> [128, KC, N] partition-major
        Cd, N = a2d.shape
        return np.ascontiguousarray(
            a2d.reshape(KC, 128, N).transpose(1, 0, 2))

    xTs = [swiz(x[b].T.astype(np.float16)) for b in range(B)]
    eTs = [swiz(e[b].T.astype(np.float16)) for b in range(B)]
    in_maps = []
    for c in range(NCORES):
        b = c // 4
        h0 = (c % 4) * HPC
        cs = h0 * HD
        w4 = {}
        for nm, W in (("Wq4", Wq), ("Wk4", Wk), ("Wv4", Wv)):
            w4[nm] = swiz(W[h0:h0 + HPC].transpose(1, 0, 2)
                          .reshape(C, HPC * HD).astype(np.float16))
        b6 = np.stack([bq[h0:h0 + HPC].reshape(2, 128),
                       bk[h0:h0 + HPC].reshape(2, 128),
                       bv[h0:h0 + HPC].reshape(2, 128)])  # [3, 2, 128]
        b6 = np.ascontiguousarray(
            b6.reshape(6, 128).T.astype(np.float32))      # [128, 6]
        bvrow = np.ascontiguousarray(
            bv[h0:h0 + HPC].reshape(1, 256).astype(np.float16))
        wpt = np.ascontiguousarray(
            Wp[:, cs:cs + HPC * HD].T.astype(np.float16)
            .reshape(2, 128, C).transpose(1, 0, 2))       # [128, 2, C]
        in_maps.append({
            "xT": xTs[b], "eT": eTs[b],
            "Wq4": w4["Wq4"], "Wk4": w4["Wk4"], "Wv4": w4["Wv4"],
            "b6": b6, "bvr": bvrow, "WpT4": wpt,
        })
    return in_maps


def kernel(e, x, Wq, bq, Wk, bk, Wv, bv, Wp):
    global LAST_RESULTS
    nc = _get_nc()
    in_maps = make_in_maps(e, x, Wq, bq, Wk, bk, Wv, bv, Wp)
    res = run_bass_kernel_spmd(
        nc, in_maps, list(range(NCORES)),
        trace=bool(os.environ.get("BASS_TRACE")),
    )
    LAST_RESULTS = res
    out = np.zeros((B, T, C), dtype=np.float32)
    for c in range(NCORES):
        out[c // 4] += res.results[c]["y"].astype(np.float32)
    return out


# revision 41
# speedup vs baseline: 1.0014x; 1.0014x over previous
"""Multi-head cross-attention Trainium2 kernel (8-core SPMD).

Sharding: 2 batch groups x 4 cores. Core c handles batch b = c // 4 and
heads [4*(c%4), 4*(c%4)+4). Each core computes its 4 heads' attention
output and a partial output projection (row-sharded Wp); the host sums
the 4 partials per batch (the all-reduce step of tensor parallelism).

All matmul operands are fp16 (PE runs fp16 at full rate), accumulation
fp32 in PSUM. Structure (per core, "mh" = head pair, 2 per core):

  Q^T[d,t] = Wq4.T @ xT (+bias)
  K^T[d,s] = Wk4.T @ eT (+bias)   -- lhsT for the QK matmuls
  V[s,d]   = eT.T @ Wv4 (+bias via a K=1 ones-row matmul) -- computed
             directly in [s,d] layout (no PE transposes); DVE strided
             copy drops it into VA[s, v0|1|v1|1] (softmax-denominator
             ones columns ride along).
  attention loop (per mh, tq=512 queries, s-tile=128 keys):
    att[s, h0q|h1q] = joint [128,1024] PSUM tile; the two QK matmuls
    (K=64 each) auto-row-tile into array halves and run concurrently.
    exp: ACT (exact) for 14/16 s-tiles; DVE Schraudolph fp16-bitcast
    (i16 = att*A + B) for s in {3, 11} to keep ACT off the critical
    path. U_aug[65,tq] += V_aug.T @ E accumulates outputs + denoms.
  normalize: denom row -> fp16 -> PE ones-broadcast -> fast reciprocal
    -> UN = U * (1/d); deferred into the next tq's PE slack.
  outproj: per t-tile, 4 accumulating matmuls -> fp16 evac -> DMA.
    y is fp16; the host sums partials in fp32.

Deferred work (next pair's Q/K, normalize, outproj) is paced into the
attention loop with per-iteration PE/DVE cost budgets; during (mh0,
tq0) the K-chunks and V-tiles stream in just ahead of their first use
so attention starts as soon as the first chunks land (~17us).

PSUM: att 2x2 banks + uh 2x1 + kv 2x1 (Q/K/V psums, denom bcast,
outproj all share the kv tag) = 8 banks.
"""

import os
import numpy as np
from contextlib import ExitStack
from collections import deque

import concourse.bass as bass
import concourse.bacc as bacc
import concourse.tile as tile
from concourse import mybir
from concourse.bass_utils import run_bass_kernel_spmd

F32 = mybir.dt.float32
F16 = mybir.dt.float16
I16 = mybir.dt.int16
AF = mybir.ActivationFunctionType
ALU = mybir.AluOpType

B, T, S, C = 2, 2048, 2048, 1024
H, HD = 16, 64
NCORES = 8
HPC = 4            # heads per core
MHN = 2            # head-pairs per core
KC = C // 128      # 8 contraction tiles
STILES = S // 128  # 16
TTILES = T // 128  # 16
TQN = 4            # t-quarters of 512
SCALE = 1.0 / np.sqrt(C)

# Schraudolph fp16 exp on DVE: i16 = round(att*EXP_A + EXP_B) bitcast
# to fp16 ~= exp(att*SCALE). C-shift -24 centers the mantissa-linear
# error (RMS ~1.05%). Only the s-tiles in DVE_EXP_TILES take this path.
EXP_A = float(1024.0 * np.log2(np.e) * SCALE)
EXP_B = float(15 * 1024 - 24)
DVE_EXP_TILES = (3, 11)

LAST_RESULTS = None
_NC_CACHE = None


def _build_nc():
    nc = bacc.Bacc()

    # all inputs host-pre-swizzled to partition-major [128, ...] layouts
    xT = nc.declare_dram_parameter("xT", [128, KC, T], F16, isOutput=False)
    eT = nc.declare_dram_parameter("eT", [128, KC, S], F16, isOutput=False)
    Wq4 = nc.declare_dram_parameter("Wq4", [128, KC, 256], F16, isOutput=False)
    Wk4 = nc.declare_dram_parameter("Wk4", [128, KC, 256], F16, isOutput=False)
    Wv4 = nc.declare_dram_parameter("Wv4", [128, KC, 256], F16, isOutput=False)
    b6 = nc.declare_dram_parameter("b6", [128, 6], F32, isOutput=False)
    bvr = nc.declare_dram_parameter("bvr", [1, 256], F16, isOutput=False)
    WpT4 = nc.declare_dram_parameter("WpT4", [128, 2, C], F16, isOutput=False)
    y = nc.declare_dram_parameter("y", [T, C], F16, isOutput=True)


    with tile.TileContext(nc) as tc, ExitStack() as ctx:
        consts = ctx.enter_context(tc.tile_pool(name="consts", bufs=1))
        wpool = ctx.enter_context(tc.tile_pool(name="wts", bufs=1))
        qkvp = ctx.enter_context(tc.tile_pool(name="qkvt", bufs=2))
        vap = ctx.enter_context(tc.tile_pool(name="vaug", bufs=2))
        epool = ctx.enter_context(tc.tile_pool(name="esb", bufs=6))
        unp = ctx.enter_context(tc.tile_pool(name="unorm", bufs=2))
        usbp = ctx.enter_context(tc.tile_pool(name="usb", bufs=3))
        dnp = ctx.enter_context(tc.tile_pool(name="denom", bufs=2))
        ysbp = ctx.enter_context(tc.tile_pool(name="ysb", bufs=3))
        psp = ctx.enter_context(tc.tile_pool(name="ps", bufs=2, space="PSUM"))

        # ---- constants ----
        # ones65[64:65, :] is the denom-broadcast lhsT; it lives on
        # partition 64 to match the usb denom row's base partition.
        ones65 = consts.tile([65, 64], F16, tag="ones65", name="ones65")
        nc.gpsimd.memset(ones65, 1.0)
        onesr = consts.tile([1, 128], F16, tag="onesr", name="onesr")
        nc.gpsimd.memset(onesr, 1.0)
        b6sb = consts.tile([128, 6], F32, tag="b6", name="b6sb")
        nc.sync.dma_start(out=b6sb, in_=b6[:, :])
        bvsb = consts.tile([1, 256], F16, tag="bvr", name="bvsb")
        nc.sync.dma_start(out=bvsb, in_=bvr[:, :])
        bsb = {"q": b6sb[:, 0:2], "k": b6sb[:, 2:4]}

        # ---- input DMAs, ordered by first consumer ----
        wsb = {}
        wsb["q"] = wpool.tile([128, KC, 256], F16, tag="wq", name="wqsb")
        nc.sync.dma_start(out=wsb["q"], in_=Wq4[:, :, :])
        xt_sb = wpool.tile([128, KC, T], F16, tag="xt")
        for k in range(KC):
            nc.sync.dma_start(out=xt_sb[:, k, :], in_=xT[:, k, :])
        for nm, dram in (("k", Wk4), ("v", Wv4)):
            t_ = wpool.tile([128, KC, 256], F16, tag=f"w{nm}", name=f"w{nm}sb")
            nc.sync.dma_start(out=t_, in_=dram[:, :, :])
            wsb[nm] = t_
        # eT lands s-chunk-major so K/V s-tiles can start early
        et_sb = wpool.tile([128, KC, S], F16, tag="et")
        for sc in range(4):
            csl = slice(sc * 512, (sc + 1) * 512)
            for k in range(KC):
                nc.sync.dma_start(out=et_sb[:, k, csl], in_=eT[:, k, csl])
        wpt = wpool.tile([128, 2, C], F16, tag="wpt")
        nc.sync.dma_start(out=wpt, in_=WpT4[:, :, :])

        QT = [qkvp.tile([128, T], F16, tag="qt", name=f"QT{i}")
              for i in range(MHN)]
        KT = [qkvp.tile([128, S], F16, tag="kt", name=f"KT{i}")
              for i in range(MHN)]
        VA = [vap.tile([128, STILES, 130], F16, tag="va", name=f"va{i}")
              for i in range(MHN)]
        for i in range(MHN):
            nc.gpsimd.memset(VA[i][:, :, 64:65], 1.0)
            nc.gpsimd.memset(VA[i][:, :, 129:130], 1.0)
        UN = [unp.tile([128, T], F16, tag="un", name=f"UN{i}")
              for i in range(MHN)]

        # Work items are (pe_ns, dve_ns, closure) for budgeted pacing.
        def q_pass(mh, half):
            tqs = (half * 2, half * 2 + 1)
            state = {}

            def mk_alloc():
                state["ps"] = {tq: psp.tile([128, 512], F32, tag="kv",
                                            name=f"qtps{mh}_{tq}")
                               for tq in tqs}

            yield (0, 0, mk_alloc)
            for k in range(KC):
                def mk_k(k=k):
                    lhsT = wsb["q"][:, k, mh * 128:(mh + 1) * 128]
                    for tq in tqs:
                        nc.tensor.matmul(state["ps"][tq], lhsT,
                                         xt_sb[:, k, tq * 512:(tq + 1) * 512],
                                         start=(k == 0), stop=(k == KC - 1))

                yield (430, 0, mk_k)

            def mk_evac():
                for tq in tqs:
                    nc.vector.tensor_scalar_add(
                        out=QT[mh][:, tq * 512:(tq + 1) * 512],
                        in0=state["ps"][tq], scalar1=bsb["q"][:, mh:mh + 1])

            yield (0, 1450, mk_evac)

        def k_work(mh, sc):
            csl = slice(sc * 512, (sc + 1) * 512)
            state = {}

            def mk_alloc():
                state["ps"] = psp.tile([128, 512], F32, tag="kv",
                                       name=f"ktps{mh}_{sc}")

            yield (0, 0, mk_alloc)

            for k in range(KC):
                def mk_k(k=k):
                    nc.tensor.matmul(state["ps"],
                                     wsb["k"][:, k, mh * 128:(mh + 1) * 128],
                                     et_sb[:, k, csl],
                                     start=(k == 0), stop=(k == KC - 1))

                yield (215, 0, mk_k)

            def mk_evac():
                nc.vector.tensor_scalar_add(out=KT[mh][:, csl],
                                            in0=state["ps"],
                                            scalar1=bsb["k"][:, mh:mh + 1])

            yield (0, 730, mk_evac)

        def v_work(j):
            """V[s,d] for s-tile j, all 4 heads at once; bias rides as a
            K=1 accumulating matmul; strided copy into VA (skips the
            ones columns)."""
            state = {}

            def mk_k4(k0):
                if k0 == 0:
                    state["ps"] = psp.tile([128, 512], F32, tag="kv",
                                           name=f"vps{j}")
                vp = state["ps"][:, 0:256]
                for k in range(k0, k0 + 4):
                    nc.tensor.matmul(vp, et_sb[:, k, j * 128:(j + 1) * 128],
                                     wsb["v"][:, k, 0:256],
                                     start=(k == 0), stop=False)
                if k0 == 4:
                    nc.tensor.matmul(vp, onesr, bvsb[:, :],
                                     start=False, stop=True)

            def mk_evac():
                vp = state["ps"]
                for mh in range(MHN):
                    dst = VA[mh][:, j, 0:130].rearrange(
                        "p (b d) -> p b d", b=2)[:, :, 0:64]
                    src = vp[:, mh * 128:(mh + 1) * 128].rearrange(
                        "p (b d) -> p b d", b=2)
                    nc.vector.tensor_copy(dst, src)

            yield (440, 0, lambda: mk_k4(0))
            yield (550, 0, lambda: mk_k4(4))
            yield (0, 470, mk_evac)

        y_r = y.rearrange("(tt p) o -> tt p o", p=128)

        def outproj_work(tq):
            for j in range(4):
                t = tq * 4 + j
                for n in range(2):
                    def mk(t=t, n=n):
                        nsl = slice(n * 512, (n + 1) * 512)
                        y_ps = psp.tile([128, 512], F32, tag="kv",
                                        name=f"yps{t}_{n}")
                        for mh in range(MHN):
                            nc.tensor.matmul(
                                y_ps, UN[mh][:, t * 128:(t + 1) * 128],
                                wpt[:, mh, nsl],
                                start=(mh == 0), stop=(mh == MHN - 1))
                        ysb = ysbp.tile([128, 512], F16, tag="ysb",
                                        name=f"ysb{t}_{n}")
                        nc.vector.tensor_copy(ysb, y_ps)
                        nc.sync.dma_start(out=y_r[t][:, nsl], in_=ysb)

                    yield (440, 730, mk)

        def normalize_work(mh, tq, uh0, uh1):
            """Evac U, broadcast the denom row (rides in uh row 64) via a
            fp16 ones matmul, reciprocal, divide."""
            qsl = slice(tq * 512, (tq + 1) * 512)
            state = {}

            def mk_evac():
                usb = usbp.tile([65, 1024], F16, tag="usb",
                                name=f"usb{mh}_{tq}")
                nc.vector.tensor_copy(usb[:, 0:512], uh0)
                nc.vector.tensor_copy(usb[:, 512:1024], uh1)
                state["usb"] = usb

            def mk_bcast():
                usb = state["usb"]
                bc = [psp.tile([128, 512], F32, tag="kv",
                               name=f"bcps{mh}_{tq}_{i}") for i in range(2)]
                state["bc"] = bc
                for i in range(2):
                    nc.tensor.matmul(bc[i][0:64, :], ones65[64:65, :],
                                     usb[64:65, i * 512:(i + 1) * 512],
                                     start=True, stop=True)

            def mk_div():
                usb, bc = state["usb"], state["bc"]
                rbc = dnp.tile([64, 1024], F32, tag="rbc", bufs=2,
                               name=f"rbc{mh}_{tq}")
                nc.vector.reciprocal_approx_fast(rbc[:, 0:512], bc[0][0:64, :])
                nc.vector.reciprocal_approx_fast(rbc[:, 512:1024],
                                                 bc[1][0:64, :])
                nc.vector.tensor_mul(UN[mh][0:64, qsl], usb[0:64, 0:512],
                                     rbc[:, 0:512])
                tmp1 = dnp.tile([64, 512], F16, tag="tmp1", bufs=2,
                                name=f"tmp1_{mh}_{tq}")
                nc.vector.tensor_mul(tmp1, usb[0:64, 512:1024],
                                     rbc[:, 512:1024])
                nc.gpsimd.dma_start(out=UN[mh][64:128, qsl], in_=tmp1)

            yield (0, 1300, mk_evac)
            yield (430, 0, mk_bcast)
            yield (0, 2600, mk_div)

        fast = deque()
        bulk = deque()
        stream = deque()

        def attention(mh):
            for tq in range(TQN):
                qsl = slice(tq * 512, (tq + 1) * 512)
                uh0 = psp.tile([65, 512], F32, tag="uh", name=f"uh0_{mh}_{tq}")
                uh1 = psp.tile([65, 512], F32, tag="uh", name=f"uh1_{mh}_{tq}")
                prev_av = None
                for s in range(STILES):
                    att = psp.tile([128, 1024], F32, tag="att",
                                   name=f"att_{mh}_{tq}_{s}")
                    ssl = slice(s * 128, (s + 1) * 128)
                    nc.tensor.matmul(att[:, 0:512], KT[mh][0:64, ssl],
                                     QT[mh][0:64, qsl], start=True, stop=True)
                    nc.tensor.matmul(att[:, 512:1024], KT[mh][64:128, ssl],
                                     QT[mh][64:128, qsl], start=True, stop=True)
                    ej = epool.tile([128, 1024], F16, tag="e",
                                    name=f"e_{mh}_{tq}_{s}")
                    if s in DVE_EXP_TILES:
                        nc.vector.tensor_scalar(
                            out=ej[:, :].bitcast(I16), in0=att,
                            scalar1=EXP_A, scalar2=EXP_B,
                            op0=ALU.mult, op1=ALU.add)
                    else:
                        nc.scalar.activation(ej, att, AF.Exp,
                                             scale=float(SCALE))
                    if prev_av is not None:
                        prev_av()
                    # paced deferred work: stream items (first tq only)
                    # hard-popped; otherwise budgeted fast/bulk pops
                    if stream:
                        npop = 0
                        while stream and npop < 6:
                            stream.popleft()[2]()
                            npop += 1
                    else:
                        pe_c = dve_c = npop = 0
                        while fast or bulk:
                            q = fast if fast else bulk
                            p, d, fn = q[0]
                            if npop and (pe_c + p > 460 or dve_c + d > 750):
                                break
                            q.popleft()
                            fn()
                            pe_c += p
                            dve_c += d
                            npop += 1

                    def mk_av(s=s, ej=ej, uh0=uh0, uh1=uh1):
                        nc.tensor.matmul(uh0, VA[mh][:, s, 0:65], ej[:, 0:512],
                                         start=(s == 0), stop=(s == STILES - 1))
                        nc.tensor.matmul(uh1, VA[mh][:, s, 65:130],
                                         ej[:, 512:1024],
                                         start=(s == 0), stop=(s == STILES - 1))
                    prev_av = mk_av
                prev_av()
                ngen = normalize_work(mh, tq, uh0, uh1)
                next(ngen)[2]()  # usb evac inline: frees uh slots promptly
                fast.extend(ngen)
                if mh == MHN - 1:
                    fast.extend(outproj_work(tq))

        # ---- schedule ----
        # Eager: Q pair 0 half 0 (tq0/tq1), first K chunk, first 2 V
        # tiles. The rest of KV streams into (mh0, tq0)'s loop just
        # ahead of first use; mh1's Q/K drains later as bulk.
        for it in q_pass(0, 0):
            it[2]()
        for it in k_work(0, 0):
            it[2]()
        for j in (0, 1):
            for it in v_work(j):
                it[2]()
        stream.extend(v_work(2))
        stream.extend(v_work(3))
        stream.extend(k_work(0, 1))
        for j in (4, 5):
            stream.extend(v_work(j))
        stream.extend(k_work(0, 2))
        for j in (6, 7, 8, 9):
            stream.extend(v_work(j))
        stream.extend(k_work(0, 3))
        for j in range(10, 16):
            stream.extend(v_work(j))

        bulk.extend(q_pass(0, 1))
        bulk.extend(q_pass(1, 0))
        bulk.extend(q_pass(1, 1))
        for sc in range(4):
            bulk.extend(k_work(1, sc))

        attention(0)
        attention(1)
        for q in (stream, fast, bulk):
            while q:
                q.popleft()[2]()

    nc.compile()
    return nc


def _get_nc():
    global _NC_CACHE
    if _NC_CACHE is None:
        _NC_CACHE = _build_nc()
    return _NC_CACHE


def make_in_maps(e, x, Wq, bq, Wk, bk, Wv, bv, Wp):
    e = np.asarray(e, dtype=np.float32)
    x = np.asarray(x, dtype=np.float32)
    Wq, bq = np.asarray(Wq, np.float32), np.asarray(bq, np.float32)
    Wk, bk = np.asarray(Wk, np.float32), np.asarray(bk, np.float32)
    Wv, bv = np.asarray(Wv, np.float32), np.asarray(bv, np.float32)
    Wp = np.asarray(Wp, np.float32)

    def swiz(a2d):  # [C, N] -> [128, KC, N] partition-major
        Cd, N = a2d.shape
        return np.ascontiguousarray(
            a2d.reshape(KC, 128, N).transpose(1, 0, 2))

    xTs = [swiz(x[b].T.astype(np.float16)) for b in range(B)]
    eTs = [swiz(e[b].T.astype(np.float16)) for b in range(B)]
    in_maps = []
    for c in range(NCORES):
        b = c // 4
        h0 = (c % 4) * HPC
        cs = h0 * HD
        w4 = {}
        for nm, W in (("Wq4", Wq), ("Wk4", Wk), ("Wv4", Wv)):
            w4[nm] = swiz(W[h0:h0 + HPC].transpose(1, 0, 2)
                          .reshape(C, HPC * HD).astype(np.float16))
        b6 = np.stack([bq[h0:h0 + HPC].reshape(2, 128),
                       bk[h0:h0 + HPC].reshape(2, 128),
                       bv[h0:h0 + HPC].reshape(2, 128)])  # [3, 2, 128]
        b6 = np.ascontiguousarray(
            b6.reshape(6, 128).T.astype(np.float32))      # [128, 6]
        bvrow = np.ascontiguousarray(
            bv[h0:h0 + HPC].reshape(1, 256).astype(np.float16))
        wpt = np.ascontiguousarray(
            Wp[:, cs:cs + HPC * HD].T.astype(np.float16)
            .reshape(2, 128, C).transpose(1, 0, 2))       # [128, 2, C]
        in_maps.append({
            "xT": xTs[b], "eT": eTs[b],
            "Wq4": w4["Wq4"], "Wk4": w4["Wk4"], "Wv4": w4["Wv4"],
            "b6": b6, "bvr": bvrow, "WpT4": wpt,
        })
    return in_maps


def kernel(e, x, Wq, bq, Wk, bk, Wv, bv, Wp):
    global LAST_RESULTS
    nc = _get_nc()
    in_maps = make_in_maps(e, x, Wq, bq, Wk, bk, Wv, bv, Wp)
    res = run_bass_kernel_spmd(
        nc, in_maps, list(range(NCORES)),
        trace=bool(os.environ.get("BASS_TRACE")),
    )
    LAST_RESULTS = res
    out = np.zeros((B, T, C), dtype=np.float32)
    for c in range(NCORES):
        out[c // 4] += res.results[c]["y"].astype(np.float32)
    return out


# revision 45
# speedup vs baseline: 1.0259x; 1.0245x over previous
"""Multi-head cross-attention Trainium2 kernel (8-core SPMD).

Sharding: 2 batch groups x 4 cores. Core c handles batch b = c // 4 and
heads [4*(c%4), 4*(c%4)+4). Each core computes its 4 heads' attention
output and a partial output projection (row-sharded Wp); the host sums
the 4 partials per batch (the all-reduce step of tensor parallelism).

All matmul operands are fp16 (PE runs fp16 at full rate), accumulation
fp32 in PSUM. Structure (per core, "mh" = head pair, 2 per core):

  Q^T[d,t] = Wq4.T @ xT (+bias)
  K^T[d,s] = Wk4.T @ eT (+bias)   -- lhsT for the QK matmuls
  V[s,d]   = eT.T @ Wv4 (+bias via a K=1 ones-row matmul) -- computed
             directly in [s,d] layout (no PE transposes); DVE strided
             copy drops it into VA[s, v0|1|v1|1] (softmax-denominator
             ones columns ride along).
  attention loop (per mh, tq=512 queries, s-tile=128 keys):
    att[s, h0q|h1q] = joint [128,1024] PSUM tile; the two QK matmuls
    (K=64 each) auto-row-tile into array halves and run concurrently.
    exp: ACT (exact) for 14/16 s-tiles; DVE Schraudolph fp16-bitcast
    (i16 = att*A + B) for s in {3, 11} to keep ACT off the critical
    path. U_aug[65,tq] += V_aug.T @ E accumulates outputs + denoms.
  normalize: denom row -> fp16 -> PE ones-broadcast -> fast reciprocal
    -> UN = U * (1/d); deferred into the next tq's PE slack.
  outproj: per t-tile, 4 accumulating matmuls -> fp16 evac -> DMA.
    y is fp16; the host sums partials in fp32.

Deferred work (next pair's Q/K, normalize, outproj) is paced into the
attention loop with per-iteration PE/DVE cost budgets; during (mh0,
tq0) the K-chunks and V-tiles stream in just ahead of their first use
so attention starts as soon as the first chunks land (~17us).

PSUM: att 2x2 banks + uh 2x1 + kv 2x1 (Q/K/V psums, denom bcast,
outproj all share the kv tag) = 8 banks.
"""

import os
import numpy as np
from contextlib import ExitStack
from collections import deque

import concourse.bass as bass
import concourse.bacc as bacc
import concourse.tile as tile
from concourse import mybir
from concourse.bass_utils import run_bass_kernel_spmd

F32 = mybir.dt.float32
F16 = mybir.dt.float16
I16 = mybir.dt.int16
AF = mybir.ActivationFunctionType
ALU = mybir.AluOpType

B, T, S, C = 2, 2048, 2048, 1024
H, HD = 16, 64
NCORES = 8
HPC = 4            # heads per core
MHN = 2            # head-pairs per core
KC = C // 128      # 8 contraction tiles
STILES = S // 128  # 16
TTILES = T // 128  # 16
TQN = 4            # t-quarters of 512
SCALE = 1.0 / np.sqrt(C)

# Schraudolph fp16 exp on DVE: i16 = round(att*EXP_A + EXP_B) bitcast
# to fp16 ~= exp(att*SCALE). C-shift -24 centers the mantissa-linear
# error (RMS ~1.05%). Only the s-tiles in DVE_EXP_TILES take this path.
EXP_A = float(1024.0 * np.log2(np.e) * SCALE)
EXP_B = float(15 * 1024 - 24)
DVE_EXP_TILES = (3, 7, 11, 15)

LAST_RESULTS = None
_NC_CACHE = None


def _build_nc():
    nc = bacc.Bacc()

    # all inputs host-pre-swizzled to partition-major [128, ...] layouts
    xT = nc.declare_dram_parameter("xT", [128, KC, T], F16, isOutput=False)
    eT = nc.declare_dram_parameter("eT", [128, KC, S], F16, isOutput=False)
    Wq4 = nc.declare_dram_parameter("Wq4", [128, KC, 256], F16, isOutput=False)
    Wk4 = nc.declare_dram_parameter("Wk4", [128, KC, 256], F16, isOutput=False)
    Wv4 = nc.declare_dram_parameter("Wv4", [128, KC, 256], F16, isOutput=False)
    b6 = nc.declare_dram_parameter("b6", [128, 6], F32, isOutput=False)
    bvr = nc.declare_dram_parameter("bvr", [1, 256], F16, isOutput=False)
    WpT4 = nc.declare_dram_parameter("WpT4", [128, 2, C], F16, isOutput=False)
    y = nc.declare_dram_parameter("y", [T, C], F16, isOutput=True)


    with tile.TileContext(nc) as tc, ExitStack() as ctx:
        consts = ctx.enter_context(tc.tile_pool(name="consts", bufs=1))
        wpool = ctx.enter_context(tc.tile_pool(name="wts", bufs=1))
        qkvp = ctx.enter_context(tc.tile_pool(name="qkvt", bufs=2))
        vap = ctx.enter_context(tc.tile_pool(name="vaug", bufs=2))
        epool = ctx.enter_context(tc.tile_pool(name="esb", bufs=6))
        unp = ctx.enter_context(tc.tile_pool(name="unorm", bufs=2))
        usbp = ctx.enter_context(tc.tile_pool(name="usb", bufs=3))
        dnp = ctx.enter_context(tc.tile_pool(name="denom", bufs=2))
        ysbp = ctx.enter_context(tc.tile_pool(name="ysb", bufs=3))
        psp = ctx.enter_context(tc.tile_pool(name="ps", bufs=2, space="PSUM"))

        # ---- constants ----
        # ones65[64:65, :] is the denom-broadcast lhsT; it lives on
        # partition 64 to match the usb denom row's base partition.
        ones65 = consts.tile([65, 64], F16, tag="ones65", name="ones65")
        nc.gpsimd.memset(ones65, 1.0)
        onesr = consts.tile([1, 128], F16, tag="onesr", name="onesr")
        nc.gpsimd.memset(onesr, 1.0)
        b6sb = consts.tile([128, 6], F32, tag="b6", name="b6sb")
        nc.sync.dma_start(out=b6sb, in_=b6[:, :])
        bvsb = consts.tile([1, 256], F16, tag="bvr", name="bvsb")
        nc.sync.dma_start(out=bvsb, in_=bvr[:, :])
        bsb = {"q": b6sb[:, 0:2], "k": b6sb[:, 2:4]}

        # ---- input DMAs, ordered by first consumer ----
        wsb = {}
        wsb["q"] = wpool.tile([128, KC, 256], F16, tag="wq", name="wqsb")
        nc.sync.dma_start(out=wsb["q"], in_=Wq4[:, :, :])
        xt_sb = wpool.tile([128, KC, T], F16, tag="xt")
        for k in range(KC):
            nc.sync.dma_start(out=xt_sb[:, k, :], in_=xT[:, k, :])
        for nm, dram in (("k", Wk4), ("v", Wv4)):
            t_ = wpool.tile([128, KC, 256], F16, tag=f"w{nm}", name=f"w{nm}sb")
            nc.sync.dma_start(out=t_, in_=dram[:, :, :])
            wsb[nm] = t_
        # eT lands s-chunk-major so K/V s-tiles can start early
        et_sb = wpool.tile([128, KC, S], F16, tag="et")
        for sc in range(4):
            csl = slice(sc * 512, (sc + 1) * 512)
            for k in range(KC):
                nc.sync.dma_start(out=et_sb[:, k, csl], in_=eT[:, k, csl])
        wpt = wpool.tile([128, 2, C], F16, tag="wpt")
        nc.sync.dma_start(out=wpt, in_=WpT4[:, :, :])

        QT = [qkvp.tile([128, T], F16, tag="qt", name=f"QT{i}")
              for i in range(MHN)]
        KT = [qkvp.tile([128, S], F16, tag="kt", name=f"KT{i}")
              for i in range(MHN)]
        VA = [vap.tile([128, STILES, 130], F16, tag="va", name=f"va{i}")
              for i in range(MHN)]
        for i in range(MHN):
            nc.gpsimd.memset(VA[i][:, :, 64:65], 1.0)
            nc.gpsimd.memset(VA[i][:, :, 129:130], 1.0)
        UN = [unp.tile([128, T], F16, tag="un", name=f"UN{i}")
              for i in range(MHN)]

        # Work items are (pe_ns, dve_ns, closure) for budgeted pacing.
        def q_pass(mh, half):
            tqs = (half * 2, half * 2 + 1)
            state = {}

            def mk_alloc():
                state["ps"] = {tq: psp.tile([128, 512], F32, tag="kv",
                                            name=f"qtps{mh}_{tq}")
                               for tq in tqs}

            yield (0, 0, mk_alloc)
            for k in range(KC):
                def mk_k(k=k):
                    lhsT = wsb["q"][:, k, mh * 128:(mh + 1) * 128]
                    for tq in tqs:
                        nc.tensor.matmul(state["ps"][tq], lhsT,
                                         xt_sb[:, k, tq * 512:(tq + 1) * 512],
                                         start=(k == 0), stop=(k == KC - 1))

                yield (430, 0, mk_k)

            def mk_evac():
                for tq in tqs:
                    nc.vector.tensor_scalar_add(
                        out=QT[mh][:, tq * 512:(tq + 1) * 512],
                        in0=state["ps"][tq], scalar1=bsb["q"][:, mh:mh + 1])

            yield (0, 1450, mk_evac)

        def k_work(mh, sc):
            csl = slice(sc * 512, (sc + 1) * 512)
            state = {}

            def mk_alloc():
                state["ps"] = psp.tile([128, 512], F32, tag="kv",
                                       name=f"ktps{mh}_{sc}")

            yield (0, 0, mk_alloc)

            for k in range(KC):
                def mk_k(k=k):
                    nc.tensor.matmul(state["ps"],
                                     wsb["k"][:, k, mh * 128:(mh + 1) * 128],
                                     et_sb[:, k, csl],
                                     start=(k == 0), stop=(k == KC - 1))

                yield (215, 0, mk_k)

            def mk_evac():
                nc.vector.tensor_scalar_add(out=KT[mh][:, csl],
                                            in0=state["ps"],
                                            scalar1=bsb["k"][:, mh:mh + 1])

            yield (0, 730, mk_evac)

        def v_work(j):
            """V[s,d] for s-tile j, all 4 heads at once; bias rides as a
            K=1 accumulating matmul; strided copy into VA (skips the
            ones columns)."""
            state = {}

            def mk_k4(k0):
                if k0 == 0:
                    state["ps"] = psp.tile([128, 512], F32, tag="kv",
                                           name=f"vps{j}")
                vp = state["ps"][:, 0:256]
                for k in range(k0, k0 + 4):
                    nc.tensor.matmul(vp, et_sb[:, k, j * 128:(j + 1) * 128],
                                     wsb["v"][:, k, 0:256],
                                     start=(k == 0), stop=False)
                if k0 == 4:
                    nc.tensor.matmul(vp, onesr, bvsb[:, :],
                                     start=False, stop=True)

            def mk_evac():
                vp = state["ps"]
                for mh in range(MHN):
                    dst = VA[mh][:, j, 0:130].rearrange(
                        "p (b d) -> p b d", b=2)[:, :, 0:64]
                    src = vp[:, mh * 128:(mh + 1) * 128].rearrange(
                        "p (b d) -> p b d", b=2)
                    nc.vector.tensor_copy(dst, src)

            yield (440, 0, lambda: mk_k4(0))
            yield (550, 0, lambda: mk_k4(4))
            yield (0, 470, mk_evac)

        y_r = y.rearrange("(tt p) o -> tt p o", p=128)

        def outproj_work(tq):
            for j in range(4):
                t = tq * 4 + j
                for n in range(2):
                    def mk(t=t, n=n):
                        nsl = slice(n * 512, (n + 1) * 512)
                        y_ps = psp.tile([128, 512], F32, tag="kv",
                                        name=f"yps{t}_{n}")
                        for mh in range(MHN):
                            nc.tensor.matmul(
                                y_ps, UN[mh][:, t * 128:(t + 1) * 128],
                                wpt[:, mh, nsl],
                                start=(mh == 0), stop=(mh == MHN - 1))
                        ysb = ysbp.tile([128, 512], F16, tag="ysb",
                                        name=f"ysb{t}_{n}")
                        nc.vector.tensor_copy(ysb, y_ps)
                        nc.sync.dma_start(out=y_r[t][:, nsl], in_=ysb)

                    yield (440, 730, mk)

        def normalize_work(mh, tq, uh0, uh1):
            """Evac U, broadcast the denom row (rides in uh row 64) via a
            fp16 ones matmul, reciprocal, divide."""
            qsl = slice(tq * 512, (tq + 1) * 512)
            state = {}

            def mk_evac():
                usb = usbp.tile([65, 1024], F16, tag="usb",
                                name=f"usb{mh}_{tq}")
                nc.vector.tensor_copy(usb[:, 0:512], uh0)
                nc.vector.tensor_copy(usb[:, 512:1024], uh1)
                state["usb"] = usb

            def mk_bcast():
                usb = state["usb"]
                bc = [psp.tile([128, 512], F32, tag="kv",
                               name=f"bcps{mh}_{tq}_{i}") for i in range(2)]
                state["bc"] = bc
                for i in range(2):
                    nc.tensor.matmul(bc[i][0:64, :], ones65[64:65, :],
                                     usb[64:65, i * 512:(i + 1) * 512],
                                     start=True, stop=True)

            def mk_div():
                usb, bc = state["usb"], state["bc"]
                rbc = dnp.tile([64, 1024], F32, tag="rbc", bufs=2,
                               name=f"rbc{mh}_{tq}")
                nc.vector.reciprocal_approx_fast(rbc[:, 0:512], bc[0][0:64, :])
                nc.vector.reciprocal_approx_fast(rbc[:, 512:1024],
                                                 bc[1][0:64, :])
                nc.vector.tensor_mul(UN[mh][0:64, qsl], usb[0:64, 0:512],
                                     rbc[:, 0:512])
                tmp1 = dnp.tile([64, 512], F16, tag="tmp1", bufs=2,
                                name=f"tmp1_{mh}_{tq}")
                nc.vector.tensor_mul(tmp1, usb[0:64, 512:1024],
                                     rbc[:, 512:1024])
                nc.gpsimd.dma_start(out=UN[mh][64:128, qsl], in_=tmp1)

            yield (0, 1300, mk_evac)
            yield (430, 0, mk_bcast)
            yield (0, 2600, mk_div)

        fast = deque()
        bulk = deque()
        stream = deque()

        def attention(mh):
            for tq in range(TQN):
                qsl = slice(tq * 512, (tq + 1) * 512)
                uh0 = psp.tile([65, 512], F32, tag="uh", name=f"uh0_{mh}_{tq}")
                uh1 = psp.tile([65, 512], F32, tag="uh", name=f"uh1_{mh}_{tq}")
                # AV issue schedule: PSUM accumulation is order-free, so
                # DVE-exp tiles' AVs defer 3 iters (their 1.2us exp would
                # otherwise stall the PE FIFO and the next ACT exp); ACT
                # tiles keep the 1-iter software pipeline. s=15's AV lands
                # in the flush, last-issued, keeping the stop flag valid.
                av_sched = {}
                for s in range(STILES):
                    att = psp.tile([128, 1024], F32, tag="att",
                                   name=f"att_{mh}_{tq}_{s}")
                    ssl = slice(s * 128, (s + 1) * 128)
                    nc.tensor.matmul(att[:, 0:512], KT[mh][0:64, ssl],
                                     QT[mh][0:64, qsl], start=True, stop=True)
                    nc.tensor.matmul(att[:, 512:1024], KT[mh][64:128, ssl],
                                     QT[mh][64:128, qsl], start=True, stop=True)
                    ej = epool.tile([128, 1024], F16, tag="e",
                                    name=f"e_{mh}_{tq}_{s}")
                    if s in DVE_EXP_TILES:
                        nc.vector.tensor_scalar(
                            out=ej[:, :].bitcast(I16), in0=att,
                            scalar1=EXP_A, scalar2=EXP_B,
                            op0=ALU.mult, op1=ALU.add)
                    else:
                        nc.scalar.activation(ej, att, AF.Exp,
                                             scale=float(SCALE))
                    for fn in av_sched.pop(s, ()):
                        fn()
                    # paced deferred work: stream items (first tq only)
                    # hard-popped; otherwise budgeted fast/bulk pops
                    if stream:
                        npop = 0
                        while stream and npop < 6:
                            stream.popleft()[2]()
                            npop += 1
                    else:
                        pe_c = dve_c = npop = 0
                        while fast or bulk:
                            q = fast if fast else bulk
                            p, d, fn = q[0]
                            if npop and (pe_c + p > 460 or dve_c + d > 750):
                                break
                            q.popleft()
                            fn()
                            pe_c += p
                            dve_c += d
                            npop += 1

                    def mk_av(s=s, ej=ej, uh0=uh0, uh1=uh1):
                        nc.tensor.matmul(uh0, VA[mh][:, s, 0:65], ej[:, 0:512],
                                         start=(s == 0), stop=(s == STILES - 1))
                        nc.tensor.matmul(uh1, VA[mh][:, s, 65:130],
                                         ej[:, 512:1024],
                                         start=(s == 0), stop=(s == STILES - 1))
                    lag = 3 if s in DVE_EXP_TILES else 1
                    av_sched.setdefault(s + lag, []).append(mk_av)
                for k in sorted(av_sched):
                    for fn in av_sched[k]:
                        fn()
                ngen = normalize_work(mh, tq, uh0, uh1)
                next(ngen)[2]()  # usb evac inline: frees uh slots promptly
                fast.extend(ngen)
                if mh == MHN - 1:
                    fast.extend(outproj_work(tq))

        # ---- schedule ----
        # Eager: Q pair 0 half 0 (tq0/tq1), first K chunk, first 2 V
        # tiles. The rest of KV streams into (mh0, tq0)'s loop just
        # ahead of first use; mh1's Q/K drains later as bulk.
        for it in q_pass(0, 0):
            it[2]()
        for it in k_work(0, 0):
            it[2]()
        for j in (0, 1):
            for it in v_work(j):
                it[2]()
        stream.extend(v_work(2))
        stream.extend(v_work(3))
        stream.extend(k_work(0, 1))
        for j in (4, 5):
            stream.extend(v_work(j))
        stream.extend(k_work(0, 2))
        for j in (6, 7, 8, 9):
            stream.extend(v_work(j))
        stream.extend(k_work(0, 3))
        for j in range(10, 16):
            stream.extend(v_work(j))

        bulk.extend(q_pass(0, 1))
        bulk.extend(q_pass(1, 0))
        bulk.extend(q_pass(1, 1))
        for sc in range(4):
            bulk.extend(k_work(1, sc))

        attention(0)
        attention(1)
        for q in (stream, fast, bulk):
            while q:
                q.popleft()[2]()

    nc.compile()
    return nc


def _get_nc():
    global _NC_CACHE
    if _NC_CACHE is None:
        _NC_CACHE = _build_nc()
    return _NC_CACHE


def make_in_maps(e, x, Wq, bq, Wk, bk, Wv, bv, Wp):
    e = np.asarray(e, dtype=np.float32)
    x = np.asarray(x, dtype=np.float32)
    Wq, bq = np.asarray(Wq, np.float32), np.asarray(bq, np.float32)
    Wk, bk = np.asarray(Wk, np.float32), np.asarray(bk, np.float32)
    Wv, bv = np.asarray(Wv, np.float32), np.asarray(bv, np.float32)
    Wp = np.asarray(Wp, np.float32)

    def swiz(a2d):  # [C, N] -> [128, KC, N] partition-major
        Cd, N = a2d.shape
        return np.ascontiguousarray(
            a2d.reshape(KC, 128, N).transpose(1, 0, 2))

    xTs = [swiz(x[b].T.astype(np.float16)) for b in range(B)]
    eTs = [swiz(e[b].T.astype(np.float16)) for b in range(B)]
    in_maps = []
    for c in range(NCORES):
        b = c // 4
        h0 = (c % 4) * HPC
        cs = h0 * HD
        w4 = {}
        for nm, W in (("Wq4", Wq), ("Wk4", Wk), ("Wv4", Wv)):
            w4[nm] = swiz(W[h0:h0 + HPC].transpose(1, 0, 2)
                          .reshape(C, HPC * HD).astype(np.float16))
        b6 = np.stack([bq[h0:h0 + HPC].reshape(2, 128),
                       bk[h0:h0 + HPC].reshape(2, 128),
                       bv[h0:h0 + HPC].reshape(2, 128)])  # [3, 2, 128]
        b6 = np.ascontiguousarray(
            b6.reshape(6, 128).T.astype(np.float32))      # [128, 6]
        bvrow = np.ascontiguousarray(
            bv[h0:h0 + HPC].reshape(1, 256).astype(np.float16))
        wpt = np.ascontiguousarray(
            Wp[:, cs:cs + HPC * HD].T.astype(np.float16)
            .reshape(2, 128, C).transpose(1, 0, 2))       # [128, 2, C]
        in_maps.append({
            "xT": xTs[b], "eT": eTs[b],
            "Wq4": w4["Wq4"], "Wk4": w4["Wk4"], "Wv4": w4["Wv4"],
            "b6": b6, "bvr": bvrow, "WpT4": wpt,
        })
    return in_maps


def kernel(e, x, Wq, bq, Wk, bk, Wv, bv, Wp):
    global LAST_RESULTS
    nc = _get_nc()
    in_maps = make_in_maps(e, x, Wq, bq, Wk, bk, Wv, bv, Wp)
    res = run_bass_kernel_spmd(
        nc, in_maps, list(range(NCORES)),
        trace=bool(os.environ.get("BASS_TRACE")),
    )
    LAST_RESULTS = res
    out = np.zeros((B, T, C), dtype=np.float32)
    for c in range(NCORES):
        out[c // 4] += res.results[c]["y"].astype(np.float32)
    return out


# revision 47
# speedup vs baseline: 1.0282x; 1.0023x over previous
"""Multi-head cross-attention Trainium2 kernel (8-core SPMD).

Sharding: 2 batch groups x 4 cores. Core c handles batch b = c // 4 and
heads [4*(c%4), 4*(c%4)+4). Each core computes its 4 heads' attention
output and a partial output projection (row-sharded Wp); the host sums
the 4 partials per batch (the all-reduce step of tensor parallelism).

All matmul operands are fp16 (PE runs fp16 at full rate), accumulation
fp32 in PSUM. Structure (per core, "mh" = head pair, 2 per core):

  Q^T[d,t] = Wq4.T @ xT (+bias)
  K^T[d,s] = Wk4.T @ eT (+bias)   -- lhsT for the QK matmuls
  V[s,d]   = eT.T @ Wv4 (+bias via a K=1 ones-row matmul) -- computed
             directly in [s,d] layout (no PE transposes); DVE strided
             copy drops it into VA[s, v0|1|v1|1] (softmax-denominator
             ones columns ride along).
  attention loop (per mh, tq=512 queries, s-tile=128 keys):
    att[s, h0q|h1q] = joint [128,1024] PSUM tile; the two QK matmuls
    (K=64 each) auto-row-tile into array halves and run concurrently.
    exp: ACT (exact) for 14/16 s-tiles; DVE Schraudolph fp16-bitcast
    (i16 = att*A + B) for s in {3, 11} to keep ACT off the critical
    path. U_aug[65,tq] += V_aug.T @ E accumulates outputs + denoms.
  normalize: denom row -> fp16 -> PE ones-broadcast -> fast reciprocal
    -> UN = U * (1/d); deferred into the next tq's PE slack.
  outproj: per t-tile, 4 accumulating matmuls -> fp16 evac -> DMA.
    y is fp16; the host sums partials in fp32.

Deferred work (next pair's Q/K, normalize, outproj) is paced into the
attention loop with per-iteration PE/DVE cost budgets; during (mh0,
tq0) the K-chunks and V-tiles stream in just ahead of their first use
so attention starts as soon as the first chunks land (~17us).

PSUM: att 2x2 banks + uh 2x1 + kv 2x1 (Q/K/V psums, denom bcast,
outproj all share the kv tag) = 8 banks.
"""

import os
import numpy as np
from contextlib import ExitStack
from collections import deque

import concourse.bass as bass
import concourse.bacc as bacc
import concourse.tile as tile
from concourse import mybir
from concourse.bass_utils import run_bass_kernel_spmd

F32 = mybir.dt.float32
F16 = mybir.dt.float16
I16 = mybir.dt.int16
AF = mybir.ActivationFunctionType
ALU = mybir.AluOpType

B, T, S, C = 2, 2048, 2048, 1024
H, HD = 16, 64
NCORES = 8
HPC = 4            # heads per core
MHN = 2            # head-pairs per core
KC = C // 128      # 8 contraction tiles
STILES = S // 128  # 16
TTILES = T // 128  # 16
TQN = 4            # t-quarters of 512
SCALE = 1.0 / np.sqrt(C)

# Schraudolph fp16 exp on DVE: i16 = round(att*EXP_A + EXP_B) bitcast
# to fp16 ~= exp(att*SCALE). C-shift -24 centers the mantissa-linear
# error (RMS ~1.05%). Only the s-tiles in DVE_EXP_TILES take this path.
EXP_A = float(1024.0 * np.log2(np.e) * SCALE)
EXP_B = float(15 * 1024 - 24)
DVE_EXP_TILES = (3, 7, 11, 15)

LAST_RESULTS = None
_NC_CACHE = None


def _build_nc():
    nc = bacc.Bacc()

    # all inputs host-pre-swizzled to partition-major [128, ...] layouts
    xT = nc.declare_dram_parameter("xT", [128, KC, T], F16, isOutput=False)
    eT = nc.declare_dram_parameter("eT", [128, KC, S], F16, isOutput=False)
    Wq4 = nc.declare_dram_parameter("Wq4", [128, KC, 256], F16, isOutput=False)
    Wk4 = nc.declare_dram_parameter("Wk4", [128, KC, 256], F16, isOutput=False)
    Wv4 = nc.declare_dram_parameter("Wv4", [128, KC, 256], F16, isOutput=False)
    b6 = nc.declare_dram_parameter("b6", [128, 6], F32, isOutput=False)
    bvr = nc.declare_dram_parameter("bvr", [1, 256], F16, isOutput=False)
    WpT4 = nc.declare_dram_parameter("WpT4", [128, 2, C], F16, isOutput=False)
    y = nc.declare_dram_parameter("y", [T, C], F16, isOutput=True)


    with tile.TileContext(nc) as tc, ExitStack() as ctx:
        consts = ctx.enter_context(tc.tile_pool(name="consts", bufs=1))
        wpool = ctx.enter_context(tc.tile_pool(name="wts", bufs=1))
        qkvp = ctx.enter_context(tc.tile_pool(name="qkvt", bufs=2))
        vap = ctx.enter_context(tc.tile_pool(name="vaug", bufs=2))
        epool = ctx.enter_context(tc.tile_pool(name="esb", bufs=6))
        unp = ctx.enter_context(tc.tile_pool(name="unorm", bufs=2))
        usbp = ctx.enter_context(tc.tile_pool(name="usb", bufs=3))
        dnp = ctx.enter_context(tc.tile_pool(name="denom", bufs=2))
        ysbp = ctx.enter_context(tc.tile_pool(name="ysb", bufs=3))
        psp = ctx.enter_context(tc.tile_pool(name="ps", bufs=2, space="PSUM"))

        # ---- constants ----
        # ones65[64:65, :] is the denom-broadcast lhsT; it lives on
        # partition 64 to match the usb denom row's base partition.
        ones65 = consts.tile([65, 64], F16, tag="ones65", name="ones65")
        nc.gpsimd.memset(ones65, 1.0)
        onesr = consts.tile([1, 128], F16, tag="onesr", name="onesr")
        nc.gpsimd.memset(onesr, 1.0)
        b6sb = consts.tile([128, 6], F32, tag="b6", name="b6sb")
        nc.sync.dma_start(out=b6sb, in_=b6[:, :])
        bvsb = consts.tile([1, 256], F16, tag="bvr", name="bvsb")
        nc.sync.dma_start(out=bvsb, in_=bvr[:, :])
        bsb = {"q": b6sb[:, 0:2], "k": b6sb[:, 2:4]}

        # ---- input DMAs, ordered by first consumer ----
        wsb = {}
        wsb["q"] = wpool.tile([128, KC, 256], F16, tag="wq", name="wqsb")
        nc.sync.dma_start(out=wsb["q"], in_=Wq4[:, :, :])
        xt_sb = wpool.tile([128, KC, T], F16, tag="xt")
        for k in range(KC):
            nc.sync.dma_start(out=xt_sb[:, k, :], in_=xT[:, k, :])
        for nm, dram in (("k", Wk4), ("v", Wv4)):
            t_ = wpool.tile([128, KC, 256], F16, tag=f"w{nm}", name=f"w{nm}sb")
            nc.sync.dma_start(out=t_, in_=dram[:, :, :])
            wsb[nm] = t_
        # eT lands s-chunk-major so K/V s-tiles can start early
        et_sb = wpool.tile([128, KC, S], F16, tag="et")
        for sc in range(4):
            csl = slice(sc * 512, (sc + 1) * 512)
            for k in range(KC):
                nc.sync.dma_start(out=et_sb[:, k, csl], in_=eT[:, k, csl])
        wpt = wpool.tile([128, 2, C], F16, tag="wpt")
        nc.sync.dma_start(out=wpt, in_=WpT4[:, :, :])

        QT = [qkvp.tile([128, T], F16, tag="qt", name=f"QT{i}")
              for i in range(MHN)]
        KT = [qkvp.tile([128, S], F16, tag="kt", name=f"KT{i}")
              for i in range(MHN)]
        VA = [vap.tile([128, STILES, 130], F16, tag="va", name=f"va{i}")
              for i in range(MHN)]
        for i in range(MHN):
            nc.gpsimd.memset(VA[i][:, :, 64:65], 1.0)
            nc.gpsimd.memset(VA[i][:, :, 129:130], 1.0)
        UN = [unp.tile([128, T], F16, tag="un", name=f"UN{i}")
              for i in range(MHN)]

        # Work items are (pe_ns, dve_ns, closure) for budgeted pacing.
        def q_pass(mh, half):
            tqs = (half * 2, half * 2 + 1)
            state = {}

            def mk_alloc():
                state["ps"] = {tq: psp.tile([128, 512], F32, tag="kv",
                                            name=f"qtps{mh}_{tq}")
                               for tq in tqs}

            yield (0, 0, mk_alloc)
            for k in range(KC):
                def mk_k(k=k):
                    lhsT = wsb["q"][:, k, mh * 128:(mh + 1) * 128]
                    for tq in tqs:
                        nc.tensor.matmul(state["ps"][tq], lhsT,
                                         xt_sb[:, k, tq * 512:(tq + 1) * 512],
                                         start=(k == 0), stop=(k == KC - 1))

                yield (430, 0, mk_k)

            def mk_evac():
                for tq in tqs:
                    nc.vector.tensor_scalar_add(
                        out=QT[mh][:, tq * 512:(tq + 1) * 512],
                        in0=state["ps"][tq], scalar1=bsb["q"][:, mh:mh + 1])

            yield (0, 1450, mk_evac)

        def k_work(mh, sc):
            csl = slice(sc * 512, (sc + 1) * 512)
            state = {}

            def mk_alloc():
                state["ps"] = psp.tile([128, 512], F32, tag="kv",
                                       name=f"ktps{mh}_{sc}")

            yield (0, 0, mk_alloc)

            for k in range(KC):
                def mk_k(k=k):
                    nc.tensor.matmul(state["ps"],
                                     wsb["k"][:, k, mh * 128:(mh + 1) * 128],
                                     et_sb[:, k, csl],
                                     start=(k == 0), stop=(k == KC - 1))

                yield (215, 0, mk_k)

            def mk_evac():
                nc.vector.tensor_scalar_add(out=KT[mh][:, csl],
                                            in0=state["ps"],
                                            scalar1=bsb["k"][:, mh:mh + 1])

            yield (0, 730, mk_evac)

        def v_work(j):
            """V[s,d] for s-tile j, all 4 heads at once; bias rides as a
            K=1 accumulating matmul; strided copy into VA (skips the
            ones columns)."""
            state = {}

            def mk_k4(k0):
                if k0 == 0:
                    state["ps"] = psp.tile([128, 512], F32, tag="kv",
                                           name=f"vps{j}")
                vp = state["ps"][:, 0:256]
                for k in range(k0, k0 + 4):
                    nc.tensor.matmul(vp, et_sb[:, k, j * 128:(j + 1) * 128],
                                     wsb["v"][:, k, 0:256],
                                     start=(k == 0), stop=False)
                if k0 == 4:
                    nc.tensor.matmul(vp, onesr, bvsb[:, :],
                                     start=False, stop=True)

            def mk_evac():
                vp = state["ps"]
                for mh in range(MHN):
                    dst = VA[mh][:, j, 0:130].rearrange(
                        "p (b d) -> p b d", b=2)[:, :, 0:64]
                    src = vp[:, mh * 128:(mh + 1) * 128].rearrange(
                        "p (b d) -> p b d", b=2)
                    nc.vector.tensor_copy(dst, src)

            yield (440, 0, lambda: mk_k4(0))
            yield (550, 0, lambda: mk_k4(4))
            yield (0, 470, mk_evac)

        y_r = y.rearrange("(tt p) o -> tt p o", p=128)

        def outproj_work(tq):
            for j in range(4):
                t = tq * 4 + j
                for n in range(2):
                    def mk(t=t, n=n):
                        nsl = slice(n * 512, (n + 1) * 512)
                        y_ps = psp.tile([128, 512], F32, tag="kv",
                                        name=f"yps{t}_{n}")
                        for mh in range(MHN):
                            nc.tensor.matmul(
                                y_ps, UN[mh][:, t * 128:(t + 1) * 128],
                                wpt[:, mh, nsl],
                                start=(mh == 0), stop=(mh == MHN - 1))
                        ysb = ysbp.tile([128, 512], F16, tag="ysb",
                                        name=f"ysb{t}_{n}")
                        nc.vector.tensor_copy(ysb, y_ps)
                        nc.sync.dma_start(out=y_r[t][:, nsl], in_=ysb)

                    yield (440, 730, mk)

        def normalize_work(mh, tq, uh0, uh1):
            """Evac U, broadcast the denom row (rides in uh row 64) via a
            fp16 ones matmul, reciprocal, divide."""
            qsl = slice(tq * 512, (tq + 1) * 512)
            state = {}

            def mk_evac():
                usb = usbp.tile([65, 1024], F16, tag="usb",
                                name=f"usb{mh}_{tq}")
                nc.vector.tensor_copy(usb[:, 0:512], uh0)
                nc.vector.tensor_copy(usb[:, 512:1024], uh1)
                state["usb"] = usb

            def mk_bcast():
                usb = state["usb"]
                bc = [psp.tile([128, 512], F32, tag="kv",
                               name=f"bcps{mh}_{tq}_{i}") for i in range(2)]
                state["bc"] = bc
                for i in range(2):
                    nc.tensor.matmul(bc[i][0:64, :], ones65[64:65, :],
                                     usb[64:65, i * 512:(i + 1) * 512],
                                     start=True, stop=True)

            def mk_div():
                usb, bc = state["usb"], state["bc"]
                rbc = dnp.tile([64, 1024], F32, tag="rbc", bufs=2,
                               name=f"rbc{mh}_{tq}")
                nc.vector.reciprocal_approx_fast(rbc[:, 0:512], bc[0][0:64, :])
                nc.vector.reciprocal_approx_fast(rbc[:, 512:1024],
                                                 bc[1][0:64, :])
                nc.vector.tensor_mul(UN[mh][0:64, qsl], usb[0:64, 0:512],
                                     rbc[:, 0:512])
                tmp1 = dnp.tile([64, 512], F16, tag="tmp1", bufs=2,
                                name=f"tmp1_{mh}_{tq}")
                nc.vector.tensor_mul(tmp1, usb[0:64, 512:1024],
                                     rbc[:, 512:1024])
                nc.gpsimd.dma_start(out=UN[mh][64:128, qsl], in_=tmp1)

            yield (0, 1300, mk_evac)
            yield (430, 0, mk_bcast)
            yield (0, 2600, mk_div)

        fast = deque()
        bulk = deque()
        stream = deque()

        def attention(mh):
            for tq in range(TQN):
                qsl = slice(tq * 512, (tq + 1) * 512)
                uh0 = psp.tile([65, 512], F32, tag="uh", name=f"uh0_{mh}_{tq}")
                uh1 = psp.tile([65, 512], F32, tag="uh", name=f"uh1_{mh}_{tq}")
                # AV issue schedule: PSUM accumulation is order-free, so
                # DVE-exp tiles' AVs defer 3 iters (their 1.2us exp would
                # otherwise stall the PE FIFO and the next ACT exp); ACT
                # tiles keep the 1-iter software pipeline. s=15's AV lands
                # in the flush, last-issued, keeping the stop flag valid.
                av_sched = {}
                for s in range(STILES):
                    att = psp.tile([128, 1024], F32, tag="att",
                                   name=f"att_{mh}_{tq}_{s}")
                    ssl = slice(s * 128, (s + 1) * 128)
                    nc.tensor.matmul(att[:, 0:512], KT[mh][0:64, ssl],
                                     QT[mh][0:64, qsl], start=True, stop=True)
                    nc.tensor.matmul(att[:, 512:1024], KT[mh][64:128, ssl],
                                     QT[mh][64:128, qsl], start=True, stop=True)
                    ej = epool.tile([128, 1024], F16, tag="e",
                                    name=f"e_{mh}_{tq}_{s}")
                    if s in DVE_EXP_TILES:
                        nc.vector.tensor_scalar(
                            out=ej[:, :].bitcast(I16), in0=att,
                            scalar1=EXP_A, scalar2=EXP_B,
                            op0=ALU.mult, op1=ALU.add)
                    else:
                        nc.scalar.activation(ej, att, AF.Exp,
                                             scale=float(SCALE))
                    for fn in av_sched.pop(s, ()):
                        fn()
                    # paced deferred work: stream items (first tq only)
                    # hard-popped; otherwise budgeted fast/bulk pops
                    if stream:
                        npop = 0
                        while stream and npop < 6:
                            stream.popleft()[2]()
                            npop += 1
                    else:
                        pe_c = dve_c = npop = 0
                        while fast or bulk:
                            q = fast if fast else bulk
                            p, d, fn = q[0]
                            if npop and (pe_c + p > 460 or dve_c + d > 750):
                                break
                            q.popleft()
                            fn()
                            pe_c += p
                            dve_c += d
                            npop += 1

                    def mk_av(s=s, ej=ej, uh0=uh0, uh1=uh1):
                        nc.tensor.matmul(uh0, VA[mh][:, s, 0:65], ej[:, 0:512],
                                         start=(s == 0), stop=(s == STILES - 1))
                        nc.tensor.matmul(uh1, VA[mh][:, s, 65:130],
                                         ej[:, 512:1024],
                                         start=(s == 0), stop=(s == STILES - 1))
                    lag = 3 if s in DVE_EXP_TILES else 1
                    av_sched.setdefault(s + lag, []).append(mk_av)
                for k in sorted(av_sched):
                    for fn in av_sched[k]:
                        fn()
                ngen = normalize_work(mh, tq, uh0, uh1)
                next(ngen)[2]()  # usb evac inline: frees uh slots promptly
                fast.extend(ngen)
                if mh == MHN - 1:
                    fast.extend(outproj_work(tq))

        # ---- schedule ----
        # Eager: Q pair 0 half 0 (tq0/tq1), first K chunk, first 2 V
        # tiles. The rest of KV streams into (mh0, tq0)'s loop just
        # ahead of first use; mh1's Q/K drains later as bulk.
        for it in q_pass(0, 0):
            it[2]()
        for it in k_work(0, 0):
            it[2]()
        for j in (0, 1):
            for it in v_work(j):
                it[2]()
        stream.extend(v_work(2))
        stream.extend(v_work(3))
        stream.extend(k_work(0, 1))
        for j in (4, 5):
            stream.extend(v_work(j))
        stream.extend(k_work(0, 2))
        for j in (6, 7, 8, 9):
            stream.extend(v_work(j))
        stream.extend(k_work(0, 3))
        for j in range(10, 16):
            stream.extend(v_work(j))

        bulk.extend(q_pass(0, 1))
        bulk.extend(q_pass(1, 0))
        bulk.extend(q_pass(1, 1))
        for sc in range(4):
            bulk.extend(k_work(1, sc))

        attention(0)
        attention(1)
        for q in (stream, fast, bulk):
            while q:
                q.popleft()[2]()

    nc.compile()
    return nc


def _get_nc():
    global _NC_CACHE
    if _NC_CACHE is None:
        _NC_CACHE = _build_nc()
    return _NC_CACHE


def make_in_maps(e, x, Wq, bq, Wk, bk, Wv, bv, Wp):
    e = np.asarray(e, dtype=np.float32)
    x = np.asarray(x, dtype=np.float32)
    Wq, bq = np.asarray(Wq, np.float32), np.asarray(bq, np.float32)
    Wk, bk = np.asarray(Wk, np.float32), np.asarray(bk, np.float32)
    Wv, bv = np.asarray(Wv, np.float32), np.asarray(bv, np.float32)
    Wp = np.asarray(Wp, np.float32)

    def swiz(a2d):  # [C, N] -> [128, KC, N] partition-major
        Cd, N = a2d.shape
        return np.ascontiguousarray(
            a2d.reshape(KC, 128, N).transpose(1, 0, 2))

    xTs = [swiz(x[b].T.astype(np.float16)) for b in range(B)]
    eTs = [swiz(e[b].T.astype(np.float16)) for b in range(B)]
    in_maps = []
    for c in range(NCORES):
        b = c // 4
        h0 = (c % 4) * HPC
        cs = h0 * HD
        w4 = {}
        for nm, W in (("Wq4", Wq), ("Wk4", Wk), ("Wv4", Wv)):
            w4[nm] = swiz(W[h0:h0 + HPC].transpose(1, 0, 2)
                          .reshape(C, HPC * HD).astype(np.float16))
        b6 = np.stack([bq[h0:h0 + HPC].reshape(2, 128),
                       bk[h0:h0 + HPC].reshape(2, 128),
                       bv[h0:h0 + HPC].reshape(2, 128)])  # [3, 2, 128]
        b6 = np.ascontiguousarray(
            b6.reshape(6, 128).T.astype(np.float32))      # [128, 6]
        bvrow = np.ascontiguousarray(
            bv[h0:h0 + HPC].reshape(1, 256).astype(np.float16))
        wpt = np.ascontiguousarray(
            Wp[:, cs:cs + HPC * HD].T.astype(np.float16)
            .reshape(2, 128, C).transpose(1, 0, 2))       # [128, 2, C]
        in_maps.append({
            "xT": xTs[b], "eT": eTs[b],
            "Wq4": w4["Wq4"], "Wk4": w4["Wk4"], "Wv4": w4["Wv4"],
            "b6": b6, "bvr": bvrow, "WpT4": wpt,
        })
    return in_maps


def kernel(e, x, Wq, bq, Wk, bk, Wv, bv, Wp):
    global LAST_RESULTS
    nc = _get_nc()
    in_maps = make_in_maps(e, x, Wq, bq, Wk, bk, Wv, bv, Wp)
    res = run_bass_kernel_spmd(
        nc, in_maps, list(range(NCORES)),
        trace=bool(os.environ.get("BASS_TRACE")),
    )
    LAST_RESULTS = res
    out = np.zeros((B, T, C), dtype=np.float32)
    for c in range(NCORES):
        out[c // 4] += res.results[c]["y"].astype(np.float32)
    return out


# revision 49
# speedup vs baseline: 1.0283x; 1.0001x over previous
"""Multi-head cross-attention Trainium2 kernel (8-core SPMD).

Sharding: 2 batch groups x 4 cores. Core c handles batch b = c // 4 and
heads [4*(c%4), 4*(c%4)+4). Each core computes its 4 heads' attention
output and a partial output projection (row-sharded Wp); the host sums
the 4 partials per batch (the all-reduce step of tensor parallelism).

All matmul operands are fp16 (PE runs fp16 at full rate), accumulation
fp32 in PSUM. Structure (per core, "mh" = head pair, 2 per core):

  Q^T[d,t] = Wq4.T @ xT (+bias)
  K^T[d,s] = Wk4.T @ eT (+bias)   -- lhsT for the QK matmuls
  V[s,d]   = eT.T @ Wv4 (+bias via a K=1 ones-row matmul) -- computed
             directly in [s,d] layout (no PE transposes); DVE strided
             copy drops it into VA[s, v0|1|v1|1] (softmax-denominator
             ones columns ride along).
  attention loop (per mh, tq=512 queries, s-tile=128 keys):
    att[s, h0q|h1q] = joint [128,1024] PSUM tile; the two QK matmuls
    (K=64 each) auto-row-tile into array halves and run concurrently.
    exp: ACT (exact) for 14/16 s-tiles; DVE Schraudolph fp16-bitcast
    (i16 = att*A + B) for s in {3, 11} to keep ACT off the critical
    path. U_aug[65,tq] += V_aug.T @ E accumulates outputs + denoms.
  normalize: denom row -> fp16 -> PE ones-broadcast -> fast reciprocal
    -> UN = U * (1/d); deferred into the next tq's PE slack.
  outproj: per t-tile, 4 accumulating matmuls -> fp16 evac -> DMA.
    y is fp16; the host sums partials in fp32.

Deferred work (next pair's Q/K, normalize, outproj) is paced into the
attention loop with per-iteration PE/DVE cost budgets; during (mh0,
tq0) the K-chunks and V-tiles stream in just ahead of their first use
so attention starts as soon as the first chunks land (~17us).

PSUM: att 2x2 banks + uh 2x1 + kv 2x1 (Q/K/V psums, denom bcast,
outproj all share the kv tag) = 8 banks.
"""

import os
import numpy as np
from contextlib import ExitStack
from collections import deque

import concourse.bass as bass
import concourse.bacc as bacc
import concourse.tile as tile
from concourse import mybir
from concourse.bass_utils import run_bass_kernel_spmd

F32 = mybir.dt.float32
F16 = mybir.dt.float16
I16 = mybir.dt.int16
AF = mybir.ActivationFunctionType
ALU = mybir.AluOpType

B, T, S, C = 2, 2048, 2048, 1024
H, HD = 16, 64
NCORES = 8
HPC = 4            # heads per core
MHN = 2            # head-pairs per core
KC = C // 128      # 8 contraction tiles
STILES = S // 128  # 16
TTILES = T // 128  # 16
TQN = 4            # t-quarters of 512
SCALE = 1.0 / np.sqrt(C)

# Schraudolph fp16 exp on DVE: i16 = round(att*EXP_A + EXP_B) bitcast
# to fp16 ~= exp(att*SCALE). C-shift -24 centers the mantissa-linear
# error (RMS ~1.05%). Only the s-tiles in DVE_EXP_TILES take this path.
EXP_A = float(1024.0 * np.log2(np.e) * SCALE)
EXP_B = float(15 * 1024 - 24)
DVE_EXP_TILES = (3, 7, 11, 15)

LAST_RESULTS = None
_NC_CACHE = None


def _build_nc():
    nc = bacc.Bacc()

    # all inputs host-pre-swizzled to partition-major [128, ...] layouts
    xT = nc.declare_dram_parameter("xT", [128, KC, T], F16, isOutput=False)
    eT = nc.declare_dram_parameter("eT", [128, KC, S], F16, isOutput=False)
    Wq4 = nc.declare_dram_parameter("Wq4", [128, KC, 256], F16, isOutput=False)
    Wk4 = nc.declare_dram_parameter("Wk4", [128, KC, 256], F16, isOutput=False)
    Wv4 = nc.declare_dram_parameter("Wv4", [128, KC, 256], F16, isOutput=False)
    b6 = nc.declare_dram_parameter("b6", [128, 6], F32, isOutput=False)
    bvr = nc.declare_dram_parameter("bvr", [1, 256], F16, isOutput=False)
    WpT4 = nc.declare_dram_parameter("WpT4", [128, 2, C], F16, isOutput=False)
    y = nc.declare_dram_parameter("y", [T, C], F16, isOutput=True)


    with tile.TileContext(nc) as tc, ExitStack() as ctx:
        consts = ctx.enter_context(tc.tile_pool(name="consts", bufs=1))
        wpool = ctx.enter_context(tc.tile_pool(name="wts", bufs=1))
        qkvp = ctx.enter_context(tc.tile_pool(name="qkvt", bufs=2))
        vap = ctx.enter_context(tc.tile_pool(name="vaug", bufs=2))
        epool = ctx.enter_context(tc.tile_pool(name="esb", bufs=6))
        unp = ctx.enter_context(tc.tile_pool(name="unorm", bufs=2))
        usbp = ctx.enter_context(tc.tile_pool(name="usb", bufs=3))
        dnp = ctx.enter_context(tc.tile_pool(name="denom", bufs=2))
        ysbp = ctx.enter_context(tc.tile_pool(name="ysb", bufs=3))
        psp = ctx.enter_context(tc.tile_pool(name="ps", bufs=2, space="PSUM"))

        # ---- constants ----
        # ones65[64:65, :] is the denom-broadcast lhsT; it lives on
        # partition 64 to match the usb denom row's base partition.
        ones65 = consts.tile([65, 64], F16, tag="ones65", name="ones65")
        nc.gpsimd.memset(ones65, 1.0)
        wrm = consts.tile([128, 128], F16, tag="wrm", name="wrm")
        nc.gpsimd.memset(wrm, 0.001)
        onesr = consts.tile([1, 128], F16, tag="onesr", name="onesr")
        nc.gpsimd.memset(onesr, 1.0)
        b6sb = consts.tile([128, 6], F32, tag="b6", name="b6sb")
        nc.sync.dma_start(out=b6sb, in_=b6[:, :])
        bvsb = consts.tile([1, 256], F16, tag="bvr", name="bvsb")
        nc.sync.dma_start(out=bvsb, in_=bvr[:, :])
        bsb = {"q": b6sb[:, 0:2], "k": b6sb[:, 2:4]}

        # ---- input DMAs, ordered by first consumer ----
        wsb = {}
        wsb["q"] = wpool.tile([128, KC, 256], F16, tag="wq", name="wqsb")
        nc.sync.dma_start(out=wsb["q"], in_=Wq4[:, :, :])
        xt_sb = wpool.tile([128, KC, T], F16, tag="xt")
        for k in range(KC):
            nc.sync.dma_start(out=xt_sb[:, k, :], in_=xT[:, k, :])
        for nm, dram in (("k", Wk4), ("v", Wv4)):
            t_ = wpool.tile([128, KC, 256], F16, tag=f"w{nm}", name=f"w{nm}sb")
            nc.sync.dma_start(out=t_, in_=dram[:, :, :])
            wsb[nm] = t_
        # eT lands s-chunk-major so K/V s-tiles can start early
        et_sb = wpool.tile([128, KC, S], F16, tag="et")
        for sc in range(4):
            csl = slice(sc * 512, (sc + 1) * 512)
            for k in range(KC):
                nc.sync.dma_start(out=et_sb[:, k, csl], in_=eT[:, k, csl])
        wpt = wpool.tile([128, 2, C], F16, tag="wpt")
        nc.sync.dma_start(out=wpt, in_=WpT4[:, :, :])

        QT = [qkvp.tile([128, T], F16, tag="qt", name=f"QT{i}")
              for i in range(MHN)]
        KT = [qkvp.tile([128, S], F16, tag="kt", name=f"KT{i}")
              for i in range(MHN)]
        VA = [vap.tile([128, STILES, 130], F16, tag="va", name=f"va{i}")
              for i in range(MHN)]
        for i in range(MHN):
            nc.gpsimd.memset(VA[i][:, :, 64:65], 1.0)
            nc.gpsimd.memset(VA[i][:, :, 129:130], 1.0)
        UN = [unp.tile([128, T], F16, tag="un", name=f"UN{i}")
              for i in range(MHN)]

        # Work items are (pe_ns, dve_ns, closure) for budgeted pacing.
        def q_pass(mh, half):
            tqs = (half * 2, half * 2 + 1)
            state = {}

            def mk_alloc():
                state["ps"] = {tq: psp.tile([128, 512], F32, tag="kv",
                                            name=f"qtps{mh}_{tq}")
                               for tq in tqs}

            yield (0, 0, mk_alloc)
            for k in range(KC):
                def mk_k(k=k):
                    lhsT = wsb["q"][:, k, mh * 128:(mh + 1) * 128]
                    for tq in tqs:
                        nc.tensor.matmul(state["ps"][tq], lhsT,
                                         xt_sb[:, k, tq * 512:(tq + 1) * 512],
                                         start=(k == 0), stop=(k == KC - 1))

                yield (430, 0, mk_k)

            def mk_evac():
                for tq in tqs:
                    nc.vector.tensor_scalar_add(
                        out=QT[mh][:, tq * 512:(tq + 1) * 512],
                        in0=state["ps"][tq], scalar1=bsb["q"][:, mh:mh + 1])

            yield (0, 1450, mk_evac)

        def k_work(mh, sc):
            csl = slice(sc * 512, (sc + 1) * 512)
            state = {}

            def mk_alloc():
                state["ps"] = psp.tile([128, 512], F32, tag="kv",
                                       name=f"ktps{mh}_{sc}")

            yield (0, 0, mk_alloc)

            for k in range(KC):
                def mk_k(k=k):
                    nc.tensor.matmul(state["ps"],
                                     wsb["k"][:, k, mh * 128:(mh + 1) * 128],
                                     et_sb[:, k, csl],
                                     start=(k == 0), stop=(k == KC - 1))

                yield (215, 0, mk_k)

            def mk_evac():
                nc.vector.tensor_scalar_add(out=KT[mh][:, csl],
                                            in0=state["ps"],
                                            scalar1=bsb["k"][:, mh:mh + 1])

            yield (0, 730, mk_evac)

        def v_work(j):
            """V[s,d] for s-tile j, all 4 heads at once; bias rides as a
            K=1 accumulating matmul; strided copy into VA (skips the
            ones columns)."""
            state = {}

            def mk_k4(k0):
                if k0 == 0:
                    state["ps"] = psp.tile([128, 512], F32, tag="kv",
                                           name=f"vps{j}")
                vp = state["ps"][:, 0:256]
                for k in range(k0, k0 + 4):
                    nc.tensor.matmul(vp, et_sb[:, k, j * 128:(j + 1) * 128],
                                     wsb["v"][:, k, 0:256],
                                     start=(k == 0), stop=False)
                if k0 == 4:
                    nc.tensor.matmul(vp, onesr, bvsb[:, :],
                                     start=False, stop=True)

            def mk_evac():
                vp = state["ps"]
                for mh in range(MHN):
                    dst = VA[mh][:, j, 0:130].rearrange(
                        "p (b d) -> p b d", b=2)[:, :, 0:64]
                    src = vp[:, mh * 128:(mh + 1) * 128].rearrange(
                        "p (b d) -> p b d", b=2)
                    nc.vector.tensor_copy(dst, src)

            yield (440, 0, lambda: mk_k4(0))
            yield (550, 0, lambda: mk_k4(4))
            yield (0, 470, mk_evac)

        y_r = y.rearrange("(tt p) o -> tt p o", p=128)

        def outproj_work(tq):
            for j in range(4):
                t = tq * 4 + j
                for n in range(2):
                    def mk(t=t, n=n):
                        nsl = slice(n * 512, (n + 1) * 512)
                        y_ps = psp.tile([128, 512], F32, tag="kv",
                                        name=f"yps{t}_{n}")
                        for mh in range(MHN):
                            nc.tensor.matmul(
                                y_ps, UN[mh][:, t * 128:(t + 1) * 128],
                                wpt[:, mh, nsl],
                                start=(mh == 0), stop=(mh == MHN - 1))
                        ysb = ysbp.tile([128, 512], F16, tag="ysb",
                                        name=f"ysb{t}_{n}")
                        nc.vector.tensor_copy(ysb, y_ps)
                        nc.sync.dma_start(out=y_r[t][:, nsl], in_=ysb)

                    yield (440, 730, mk)

        def normalize_work(mh, tq, uh0, uh1):
            """Evac U, broadcast the denom row (rides in uh row 64) via a
            fp16 ones matmul, reciprocal, divide."""
            qsl = slice(tq * 512, (tq + 1) * 512)
            state = {}

            def mk_evac():
                usb = usbp.tile([65, 1024], F16, tag="usb",
                                name=f"usb{mh}_{tq}")
                nc.vector.tensor_copy(usb[:, 0:512], uh0)
                nc.vector.tensor_copy(usb[:, 512:1024], uh1)
                state["usb"] = usb

            def mk_bcast():
                usb = state["usb"]
                bc = [psp.tile([128, 512], F32, tag="kv",
                               name=f"bcps{mh}_{tq}_{i}") for i in range(2)]
                state["bc"] = bc
                for i in range(2):
                    nc.tensor.matmul(bc[i][0:64, :], ones65[64:65, :],
                                     usb[64:65, i * 512:(i + 1) * 512],
                                     start=True, stop=True)

            def mk_div():
                usb, bc = state["usb"], state["bc"]
                rbc = dnp.tile([64, 1024], F32, tag="rbc", bufs=2,
                               name=f"rbc{mh}_{tq}")
                nc.vector.reciprocal_approx_fast(rbc[:, 0:512], bc[0][0:64, :])
                nc.vector.reciprocal_approx_fast(rbc[:, 512:1024],
                                                 bc[1][0:64, :])
                nc.vector.tensor_mul(UN[mh][0:64, qsl], usb[0:64, 0:512],
                                     rbc[:, 0:512])
                tmp1 = dnp.tile([64, 512], F16, tag="tmp1", bufs=2,
                                name=f"tmp1_{mh}_{tq}")
                nc.vector.tensor_mul(tmp1, usb[0:64, 512:1024],
                                     rbc[:, 512:1024])
                nc.gpsimd.dma_start(out=UN[mh][64:128, qsl], in_=tmp1)

            yield (0, 1300, mk_evac)
            yield (430, 0, mk_bcast)
            yield (0, 2600, mk_div)

        fast = deque()
        bulk = deque()
        stream = deque()

        def attention(mh):
            for tq in range(TQN):
                qsl = slice(tq * 512, (tq + 1) * 512)
                uh0 = psp.tile([65, 512], F32, tag="uh", name=f"uh0_{mh}_{tq}")
                uh1 = psp.tile([65, 512], F32, tag="uh", name=f"uh1_{mh}_{tq}")
                # AV issue schedule: PSUM accumulation is order-free, so
                # DVE-exp tiles' AVs defer 3 iters (their 1.2us exp would
                # otherwise stall the PE FIFO and the next ACT exp); ACT
                # tiles keep the 1-iter software pipeline. s=15's AV lands
                # in the flush, last-issued, keeping the stop flag valid.
                av_sched = {}
                for s in range(STILES):
                    att = psp.tile([128, 1024], F32, tag="att",
                                   name=f"att_{mh}_{tq}_{s}")
                    ssl = slice(s * 128, (s + 1) * 128)
                    nc.tensor.matmul(att[:, 0:512], KT[mh][0:64, ssl],
                                     QT[mh][0:64, qsl], start=True, stop=True)
                    nc.tensor.matmul(att[:, 512:1024], KT[mh][64:128, ssl],
                                     QT[mh][64:128, qsl], start=True, stop=True)
                    ej = epool.tile([128, 1024], F16, tag="e",
                                    name=f"e_{mh}_{tq}_{s}")
                    if s in DVE_EXP_TILES:
                        nc.vector.tensor_scalar(
                            out=ej[:, :].bitcast(I16), in0=att,
                            scalar1=EXP_A, scalar2=EXP_B,
                            op0=ALU.mult, op1=ALU.add)
                    else:
                        nc.scalar.activation(ej, att, AF.Exp,
                                             scale=float(SCALE))
                    for fn in av_sched.pop(s, ()):
                        fn()
                    # paced deferred work: stream items (first tq only)
                    # hard-popped; otherwise budgeted fast/bulk pops
                    if stream:
                        npop = 0
                        while stream and npop < 6:
                            stream.popleft()[2]()
                            npop += 1
                    else:
                        pe_c = dve_c = npop = 0
                        while fast or bulk:
                            q = fast if fast else bulk
                            p, d, fn = q[0]
                            if npop and (pe_c + p > 460 or dve_c + d > 750):
                                break
                            q.popleft()
                            fn()
                            pe_c += p
                            dve_c += d
                            npop += 1

                    def mk_av(s=s, ej=ej, uh0=uh0, uh1=uh1):
                        nc.tensor.matmul(uh0, VA[mh][:, s, 0:65], ej[:, 0:512],
                                         start=(s == 0), stop=(s == STILES - 1))
                        nc.tensor.matmul(uh1, VA[mh][:, s, 65:130],
                                         ej[:, 512:1024],
                                         start=(s == 0), stop=(s == STILES - 1))
                    lag = 3 if s in DVE_EXP_TILES else 1
                    av_sched.setdefault(s + lag, []).append(mk_av)
                for k in sorted(av_sched):
                    for fn in av_sched[k]:
                        fn()
                ngen = normalize_work(mh, tq, uh0, uh1)
                next(ngen)[2]()  # usb evac inline: frees uh slots promptly
                fast.extend(ngen)
                if mh == MHN - 1:
                    fast.extend(outproj_work(tq))

        # ---- schedule ----
        # PE warm-up spin: ~4.5us of dummy matmuls while the input DMAs
        # land, so the HAM clock gate releases (1.2 -> 2.4 GHz) before
        # the real Q/K/V projections start. PE is DMA-idle here anyway.
        wps = psp.tile([128, 1024], F32, tag="att", name="warmps")
        for _ in range(42):
            nc.tensor.matmul(wps[:, 0:128], wrm, wrm,
                             start=True, stop=True)
        # Eager: Q pair 0 half 0 (tq0/tq1), first K chunk, first 2 V
        # tiles. The rest of KV streams into (mh0, tq0)'s loop just
        # ahead of first use; mh1's Q/K drains later as bulk.
        for it in q_pass(0, 0):
            it[2]()
        for it in k_work(0, 0):
            it[2]()
        for j in (0, 1):
            for it in v_work(j):
                it[2]()
        stream.extend(v_work(2))
        stream.extend(v_work(3))
        stream.extend(k_work(0, 1))
        for j in (4, 5):
            stream.extend(v_work(j))
        stream.extend(k_work(0, 2))
        for j in (6, 7, 8, 9):
            stream.extend(v_work(j))
        stream.extend(k_work(0, 3))
        for j in range(10, 16):
            stream.extend(v_work(j))

        bulk.extend(q_pass(0, 1))
        bulk.extend(q_pass(1, 0))
        bulk.extend(q_pass(1, 1))
        for sc in range(4):
            bulk.extend(k_work(1, sc))

        attention(0)
        attention(1)
        for q in (stream, fast, bulk):
            while q:
                q.popleft()[2]()

    nc.compile()
    return nc


def _get_nc():
    global _NC_CACHE
    if _NC_CACHE is None:
        _NC_CACHE = _build_nc()
    return _NC_CACHE


def make_in_maps(e, x, Wq, bq, Wk, bk, Wv, bv, Wp):
    e = np.asarray(e, dtype=np.float32)
    x = np.asarray(x, dtype=np.float32)
    Wq, bq = np.asarray(Wq, np.float32), np.asarray(bq, np.float32)
    Wk, bk = np.asarray(Wk, np.float32), np.asarray(bk, np.float32)
    Wv, bv = np.asarray(Wv, np.float32), np.asarray(bv, np.float32)
    Wp = np.asarray(Wp, np.float32)

    def swiz(a2d):  # [C, N] -> [128, KC, N] partition-major
        Cd, N = a2d.shape
        return np.ascontiguousarray(
            a2d.reshape(KC, 128, N).transpose(1, 0, 2))

    xTs = [swiz(x[b].T.astype(np.float16)) for b in range(B)]
    eTs = [swiz(e[b].T.astype(np.float16)) for b in range(B)]
    in_maps = []
    for c in range(NCORES):
        b = c // 4
        h0 = (c % 4) * HPC
        cs = h0 * HD
        w4 = {}
        for nm, W in (("Wq4", Wq), ("Wk4", Wk), ("Wv4", Wv)):
            w4[nm] = swiz(W[h0:h0 + HPC].transpose(1, 0, 2)
                          .reshape(C, HPC * HD).astype(np.float16))
        b6 = np.stack([bq[h0:h0 + HPC].reshape(2, 128),
                       bk[h0:h0 + HPC].reshape(2, 128),
                       bv[h0:h0 + HPC].reshape(2, 128)])  # [3, 2, 128]
        b6 = np.ascontiguousarray(
            b6.reshape(6, 128).T.astype(np.float32))      # [128, 6]
        bvrow = np.ascontiguousarray(
            bv[h0:h0 + HPC].reshape(1, 256).astype(np.float16))
        wpt = np.ascontiguousarray(
            Wp[:, cs:cs + HPC * HD].T.astype(np.float16)
            .reshape(2, 128, C).transpose(1, 0, 2))       # [128, 2, C]
        in_maps.append({
            "xT": xTs[b], "eT": eTs[b],
            "Wq4": w4["Wq4"], "Wk4": w4["Wk4"], "Wv4": w4["Wv4"],
            "b6": b6, "bvr": bvrow, "WpT4": wpt,
        })
    return in_maps


def kernel(e, x, Wq, bq, Wk, bk, Wv, bv, Wp):
    global LAST_RESULTS
    nc = _get_nc()
    in_maps = make_in_maps(e, x, Wq, bq, Wk, bk, Wv, bv, Wp)
    res = run_bass_kernel_spmd(
        nc, in_maps, list(range(NCORES)),
        trace=bool(os.environ.get("BASS_TRACE")),
    )
    LAST_RESULTS = res
    out = np.zeros((B, T, C), dtype=np.float32)
    for c in range(NCORES):
        out[c // 4] += res.results[c]["y"].astype(np.float32)
    return out
